# revision 1
# baseline (speedup 1.0000x reference)
# DDSP synthesizer kernel for Trainium2 (8 NeuronCores, batch-parallel).
#
# Per core (one batch element):
#   harmonic branch: exact-phase oscillator bank via PE outer-products
#     (k x w), magic-number round + Cody-Waite-free wrap on DVE, ScalarE Sin,
#     per-frame weighted reduction over harmonics on PE.
#   noise branch: irfft+window folded into one DFT matmul, frame-wise
#     128-tap causal conv via rfft-256 as PE matmuls.
#   reverb: 16000-tap causal FIR as 126 accumulating Toeplitz matmuls.
import math
import numpy as np

import concourse.bass as bass
import concourse.bacc as bacc
import concourse.mybir as mybir
from concourse import tile
from concourse.bass_utils import run_bass_kernel_spmd

F32 = mybir.dt.float32
B, F, NH, NB = 8, 400, 100, 65
SR, BLOCK = 16000, 128
T = F * BLOCK
LOG10 = math.log(10.0)
MAGIC = 12582912.0  # 1.5 * 2**23
NCHUNK = 50         # harmonic chunks of 1024 samples
CHW = 1024

_cache = {}


def _host_constants():
    b = np.arange(NB)[:, None]
    m = np.arange(128)[None, :]
    w = np.where((b == 0) | (b == 64), 1.0, 2.0)
    Cmat = w / 128.0 * np.cos(2 * np.pi * b * m / 128.0)
    win2 = 0.5 + 0.5 * np.cos(2 * np.pi * np.arange(128) / 128.0)
    Air = (2.0 * Cmat * win2[None, :]).astype(np.float32)               # (65,128)
    ccol = (1e-7 * (Cmat * win2[None, :]).sum(0)).astype(np.float32)[:, None]  # (128,1)
    j = np.arange(128)[:, None]
    bb = np.arange(128)[None, :]
    Dcos = np.cos(2 * np.pi * j * bb / 256.0).astype(np.float32)        # (128j,128b)
    Dsin = (-np.sin(2 * np.pi * j * bb / 256.0)).astype(np.float32)
    d128 = np.cos(np.pi * np.arange(128)).astype(np.float32)[:, None]   # (128,1)
    bb2 = np.arange(128)[:, None]
    i = np.arange(128)[None, :]
    cb = np.where(bb2 == 0, 1.0, 2.0)
    ICre = (cb / 256.0 * np.cos(2 * np.pi * bb2 * i / 256.0)).astype(np.float32)
    ICim = (-2.0 / 256.0 * np.sin(2 * np.pi * bb2 * i / 256.0)).astype(np.float32)
    nyq = ((1.0 / 256.0) * np.cos(np.pi * np.arange(128))).astype(np.float32)[None, :]
    kvneg = np.zeros((1, 128), np.float32)
    kvneg[0, :NH] = -np.arange(1, NH + 1)
    eye = np.eye(128, dtype=np.float32)
    return dict(c_air=Air, c_ccol=ccol, c_dcos=Dcos, c_dsin=Dsin, c_d128=d128,
                c_icre=ICre, c_icim=ICim, c_nyq=nyq, c_kvneg=kvneg, c_eye=eye)


def _build():
    nc = bacc.Bacc(None, target_bir_lowering=False, debug=False)

    amp_d = nc.dram_tensor("amp_param", [F, NH + 1], F32, kind="ExternalInput")
    npr_d = nc.dram_tensor("noise_param", [F, NB], F32, kind="ExternalInput")
    pit_d = nc.dram_tensor("pitch", [F, 1], F32, kind="ExternalInput")
    noi_d = nc.dram_tensor("noise", [F, BLOCK], F32, kind="ExternalInput")
    rvn_d = nc.dram_tensor("reverb_noise", [SR, 1], F32, kind="ExternalInput")
    dec_d = nc.dram_tensor("decay", [1, 1], F32, kind="ExternalInput")
    wet_d = nc.dram_tensor("wet", [1, 1], F32, kind="ExternalInput")
    cst = {}
    for name, shape in [("c_air", [NB, 128]), ("c_ccol", [128, 1]),
                        ("c_dcos", [128, 128]), ("c_dsin", [128, 128]),
                        ("c_d128", [128, 1]), ("c_icre", [128, 128]),
                        ("c_icim", [128, 128]), ("c_nyq", [1, 128]),
                        ("c_kvneg", [1, 128]), ("c_eye", [128, 128])]:
        cst[name] = nc.dram_tensor(name, shape, F32, kind="ExternalInput")
    out_d = nc.dram_tensor("out", [128, F], F32, kind="ExternalOutput")

    wscr = nc.dram_tensor("wscr", [1, T], F32)
    rowscr = nc.dram_tensor("rowscr", [2, F], F32)
    hpbuf = nc.dram_tensor("hpbuf", [1, 127 + 16128 + 1], F32)

    AF = mybir.ActivationFunctionType
    OP = mybir.AluOpType
    F16 = mybir.dt.float16

    with tile.TileContext(nc) as tc:
        with tc.tile_pool(name="const", bufs=1) as cpool, \
             tc.tile_pool(name="big", bufs=1) as big, \
             tc.tile_pool(name="work", bufs=1) as work, \
             tc.tile_pool(name="chunk", bufs=3) as chk:

            # ---------- constants ----------
            t_mcol = cpool.tile([128, 1], F32)
            nc.vector.memset(t_mcol[:], MAGIC)
            t_eps = cpool.tile([128, 1], F32)
            nc.vector.memset(t_eps[:], 1e-7)
            t_b5 = cpool.tile([128, 1], F32)
            nc.vector.memset(t_b5[:], 5.0)
            t_kv = cpool.tile([1, 128], F32)
            nc.sync.dma_start(t_kv[:], cst["c_kvneg"][:])
            t_air = cpool.tile([NB, 128], F32)
            nc.sync.dma_start(t_air[:], cst["c_air"][:])
            t_ccol = cpool.tile([128, 1], F32)
            nc.sync.dma_start(t_ccol[:], cst["c_ccol"][:])
            t_dcos = cpool.tile([128, 128], F32)
            nc.sync.dma_start(t_dcos[:], cst["c_dcos"][:])
            t_dsin = cpool.tile([128, 128], F32)
            nc.sync.dma_start(t_dsin[:], cst["c_dsin"][:])
            t_d128 = cpool.tile([128, 1], F32)
            nc.sync.dma_start(t_d128[:], cst["c_d128"][:])
            t_icre = cpool.tile([128, 128], F32)
            nc.sync.dma_start(t_icre[:], cst["c_icre"][:])
            t_icim = cpool.tile([128, 128], F32)
            nc.sync.dma_start(t_icim[:], cst["c_icim"][:])
            t_nyq = cpool.tile([1, 128], F32)
            nc.sync.dma_start(t_nyq[:], cst["c_nyq"][:])
            t_eye = cpool.tile([128, 128], F32)
            nc.sync.dma_start(t_eye[:], cst["c_eye"][:])
            t_ones = cpool.tile([1, 128], F32)
            nc.vector.memset(t_ones[:], 1.0)
            t_kroi = cpool.tile([128, NH], mybir.dt.int32)
            nc.gpsimd.iota(t_kroi[:], pattern=[[1, NH]], base=1, channel_multiplier=0)
            t_krow = cpool.tile([128, NH], F32)
            nc.vector.tensor_copy(t_krow[:], t_kroi[:])
            t_jp1i = cpool.tile([128, 1], mybir.dt.int32)
            nc.gpsimd.iota(t_jp1i[:], pattern=[[1, 1]], base=1, channel_multiplier=1)
            t_jp1 = cpool.tile([128, 1], F32)
            nc.vector.tensor_copy(t_jp1[:], t_jp1i[:])
            t_ioti = cpool.tile([128, 125], mybir.dt.int32)
            nc.gpsimd.iota(t_ioti[:], pattern=[[128, 125]], base=0, channel_multiplier=1)
            t_iotf = cpool.tile([128, 125], F32)
            nc.vector.tensor_copy(t_iotf[:], t_ioti[:])

            # early loads for impulse build
            t_dec = work.tile([1, 1], F32)
            nc.sync.dma_start(t_dec[:], dec_d[:])
            t_wet = work.tile([1, 1], F32)
            nc.sync.dma_start(t_wet[:], wet_d[:])
            t_rn = work.tile([128, 125], F32)
            nc.sync.dma_start(t_rn[:], bass.AP(rvn_d, 0, [[1, 128], [128, 125]]))

            with tc.high_priority():
                # ---------- phase chain (critical path to stage B) ----------
                t_pr = work.tile([1, F], F32)
                nc.sync.dma_start(t_pr[:], bass.AP(pit_d, 0, [[1, 1], [1, F]]))
                t_p8 = work.tile([1, F], F32)
                nc.scalar.activation(t_p8[:], t_pr[:], AF.Identity,
                                     bias=t_mcol[0:1, :], scale=8.0)
                t_ph = work.tile([1, F], F32)
                nc.vector.tensor_scalar(out=t_ph[:], in0=t_p8[:], scalar1=MAGIC,
                                        scalar2=0.125, op0=OP.subtract, op1=OP.mult)
                t_pl = work.tile([1, F], F32)
                nc.vector.tensor_sub(t_pl[:], t_pr[:], t_ph[:])
                t_zr = work.tile([1, F], F32)
                nc.vector.memset(t_zr[:], 0.0)
                t_sh = work.tile([1, F + 1], F32)
                nc.vector.memset(t_sh[:, 0:1], 0.0)
                nc.vector.tensor_tensor_scan(t_sh[:, 1:F + 1], t_ph[:], t_zr[:], 0.0,
                                             OP.add, OP.add)
                t_sl = work.tile([1, F + 1], F32)
                nc.vector.memset(t_sl[:, 0:1], 0.0)
                nc.vector.tensor_tensor_scan(t_sl[:, 1:F + 1], t_pl[:], t_zr[:], 0.0,
                                             OP.add, OP.add)
                t_ts = work.tile([1, F], F32)
                nc.vector.tensor_add(t_ts[:], t_sh[:, 0:F], t_sl[:, 0:F])
                t_t2r = work.tile([1, F], F32)
                nc.scalar.activation(t_t2r[:], t_ts[:], AF.Identity,
                                     bias=t_mcol[0:1, :], scale=1.0 / 125.0)
                t_n125 = work.tile([1, F], F32)
                nc.vector.tensor_scalar(out=t_n125[:], in0=t_t2r[:], scalar1=MAGIC,
                                        scalar2=None, op0=OP.subtract)
                t_u = work.tile([1, F], F32)
                nc.vector.scalar_tensor_tensor(out=t_u[:], in0=t_n125[:], scalar=-125.0,
                                               in1=t_sh[:, 0:F], op0=OP.mult, op1=OP.add)
                t_r125 = work.tile([1, F], F32)
                nc.vector.tensor_add(t_r125[:], t_u[:], t_sl[:, 0:F])
                t_om = work.tile([1, F], F32)
                nc.vector.tensor_scalar(out=t_om[:], in0=t_r125[:], scalar1=1.0 / 125.0,
                                        scalar2=None, op0=OP.mult)
                t_cr = work.tile([1, F], F32)
                nc.vector.tensor_scalar(out=t_cr[:], in0=t_pr[:], scalar1=1.0 / 16000.0,
                                        scalar2=None, op0=OP.mult)
                nc.sync.dma_start(rowscr[0:1, :], t_om[:])
                nc.sync.dma_start(rowscr[1:2, :], t_cr[:])
                t_omb = work.tile([128, F], F32)
                nc.sync.dma_start(t_omb[:], bass.AP(rowscr, 0, [[0, 128], [1, F]]))
                t_cb = work.tile([128, F], F32)
                nc.sync.dma_start(t_cb[:], bass.AP(rowscr, F, [[0, 128], [1, F]]))
                t_wraw = work.tile([128, F], F32)
                nc.vector.scalar_tensor_tensor(out=t_wraw[:], in0=t_cb[:],
                                               scalar=t_jp1[:, :], in1=t_omb[:],
                                               op0=OP.mult, op1=OP.add)
                t_tw = work.tile([128, F], F32)
                nc.scalar.activation(t_tw[:], t_wraw[:], AF.Identity,
                                     bias=t_mcol[:, :], scale=1.0)
                t_wneg = work.tile([128, F], F32)
                nc.vector.scalar_tensor_tensor(out=t_wneg[:], in0=t_tw[:], scalar=MAGIC,
                                               in1=t_wraw[:], op0=OP.subtract,
                                               op1=OP.subtract)
                nc.sync.dma_start(bass.AP(wscr, 0, [[1, 128], [128, F]]), t_wneg[:])

            with tc.tile_pool(name="pstr", bufs=2, space="PSUM") as pstr:
                # ---------- amp params ----------
                t_at = big.tile([128, F], F32)
                nc.vector.memset(t_at[:], 0.0)
                amp_sf = []
                for g in range(4):
                    f0 = g * 100
                    t_ap = work.tile([100, NH + 1], F32, tag=f"ampchunk{g}", name=f"ampchunk{g}")
                    nc.sync.dma_start(t_ap[:], amp_d[f0:f0 + 100, :])
                    t_sf = work.tile([100, NH + 1], F32, tag=f"ampsf{g}", name=f"ampsf{g}")
                    nc.scalar.activation(t_sf[:], t_ap[:], AF.Exp, scale=-1.0)
                    amp_sf.append(t_sf)
                ns_sf = []
                for g in range(4):
                    f0 = g * 100
                    t_np = work.tile([100, NB], F32, tag=f"npchunk{g}", name=f"npchunk{g}")
                    nc.sync.dma_start(t_np[:], npr_d[f0:f0 + 100, :])
                    t_ns = work.tile([100, NB], F32, tag=f"nsf{g}", name=f"nsf{g}")
                    nc.scalar.activation(t_ns[:], t_np[:], AF.Exp, bias=t_b5[0:100, :], scale=-1.0)
                    ns_sf.append(t_ns)
                for t_sf in amp_sf + ns_sf:
                    nc.scalar.activation(t_sf[:], t_sf[:], AF.Ln, bias=1.0, scale=1.0)
                for t_sf in amp_sf + ns_sf:
                    nc.scalar.activation(t_sf[:], t_sf[:], AF.Exp, scale=-LOG10)
                for t_sf in amp_sf:
                    nc.scalar.activation(t_sf[:], t_sf[:], AF.Identity,
                                         bias=t_eps[0:100, :], scale=2.0)
                t_pcol = work.tile([100, 4], F32)
                nc.sync.dma_start(t_pcol[:], bass.AP(pit_d, 0, [[1, 100], [100, 4]]))
                for g in range(4):
                    f0 = g * 100
                    t_sf = amp_sf[g]
                    t_kp = work.tile([100, NH], F32, tag="kp")
                    nc.vector.tensor_scalar(out=t_kp[:], in0=t_krow[0:100, :],
                                            scalar1=t_pcol[:, g:g + 1], scalar2=None, op0=OP.mult)
                    t_aa = work.tile([100, NH], F32, tag="aa")
                    nc.vector.tensor_scalar(out=t_aa[:], in0=t_kp[:], scalar1=8000.0,
                                            scalar2=1e-4, op0=OP.is_lt, op1=OP.add)
                    t_am = work.tile([100, NH], F32, tag="am")
                    nc.vector.tensor_mul(t_am[:], t_sf[:, 1:NH + 1], t_aa[:])
                    t_ssum = work.tile([100, 1], F32, tag="ssum")
                    nc.vector.tensor_reduce(out=t_ssum[:], in_=t_am[:],
                                            axis=mybir.AxisListType.X, op=OP.add)
                    t_rec = work.tile([100, 1], F32, tag="rec")
                    nc.vector.reciprocal(t_rec[:], t_ssum[:])
                    t_scn = work.tile([100, 1], F32, tag="scn")
                    nc.vector.tensor_mul(t_scn[:], t_rec[:], t_sf[:, 0:1])
                    nc.vector.tensor_scalar(out=t_scn[:], in0=t_scn[:], scalar1=-1.0,
                                            scalar2=None, op0=OP.mult)
                    nc.vector.tensor_scalar(out=t_am[:], in0=t_am[:], scalar1=t_scn[:, :],
                                            scalar2=None, op0=OP.mult)
                    p_tr = pstr.tile([100, 100], F32, tag="tr")
                    nc.tensor.transpose(p_tr[:], t_am[:], t_eye[0:100, 0:100])
                    nc.scalar.copy(t_at[0:100, f0:f0 + 100], p_tr[:])
                t_at16 = big.tile([128, F], F16)
                nc.vector.tensor_copy(t_at16[:], t_at[:])

                # ---------- noise branch ----------
                t_nt = big.tile([65, F], F32)
                for g in range(4):
                    f0 = g * 100
                    p_tr2 = pstr.tile([65, 100], F32, tag="tr")
                    nc.tensor.transpose(p_tr2[:], ns_sf[g][:], t_eye[0:100, 0:100])
                    nc.scalar.copy(t_nt[0:65, f0:f0 + 100], p_tr2[:])
                t_noT = big.tile([128, F], F32)
                nc.sync.dma_start(t_noT[:], bass.AP(noi_d, 0, [[1, 128], [128, F]]))

                with tc.tile_pool(name="psn", bufs=1, space="PSUM") as psn:
                    p_irp = psn.tile([128, F], F32, tag="tmp", bufs=2)
                    nc.tensor.matmul(p_irp[:], t_air[:], t_nt[0:65, :], start=True, stop=True)
                    t_irp = big.tile([128, F], F32)
                    nc.vector.tensor_scalar(out=t_irp[:], in0=p_irp[:],
                                            scalar1=t_ccol[:, :], scalar2=None, op0=OP.add)
                    p_hre = psn.tile([128, F], F32, tag="tmp", bufs=2)
                    nc.tensor.matmul(p_hre[:], t_dcos[:], t_irp[:], start=True, stop=True)
                    t_hre = big.tile([128, F], F32)
                    nc.scalar.copy(t_hre[:], p_hre[:])
                    p_him = psn.tile([128, F], F32, tag="tmp", bufs=2)
                    nc.tensor.matmul(p_him[:], t_dsin[:], t_irp[:], start=True, stop=True)
                    t_him = big.tile([128, F], F32)
                    nc.scalar.copy(t_him[:], p_him[:])
                    p_h128 = psn.tile([1, F], F32, tag="tmp", bufs=2)
                    nc.tensor.matmul(p_h128[:], t_d128[:], t_irp[:], start=True, stop=True)
                    t_h128 = big.tile([1, F], F32)
                    nc.scalar.copy(t_h128[:], p_h128[:])
                    p_nre = psn.tile([128, F], F32, tag="nre")
                    nc.tensor.matmul(p_nre[:], t_dcos[:], t_noT[:], start=True, stop=True)
                    p_nim = psn.tile([128, F], F32, tag="nim")
                    nc.tensor.matmul(p_nim[:], t_dsin[:], t_noT[:], start=True, stop=True)
                    p_n128 = psn.tile([1, F], F32, tag="tmp", bufs=2)
                    nc.tensor.matmul(p_n128[:], t_d128[:], t_noT[:], start=True, stop=True)
                    t_a = work.tile([128, F], F32, tag="pa")
                    nc.vector.tensor_mul(t_a[:], t_hre[:], p_nre[:])
                    t_b = work.tile([128, F], F32, tag="pb")
                    nc.vector.tensor_mul(t_b[:], t_him[:], p_nim[:])
                    t_pre = big.tile([128, F], F32)
                    nc.vector.tensor_sub(t_pre[:], t_a[:], t_b[:])
                    t_c2 = work.tile([128, F], F32, tag="pc2")
                    nc.vector.tensor_mul(t_c2[:], t_him[:], p_nre[:])
                    t_d2 = work.tile([128, F], F32, tag="pd2")
                    nc.vector.tensor_mul(t_d2[:], t_hre[:], p_nim[:])
                    t_pim = big.tile([128, F], F32)
                    nc.vector.tensor_add(t_pim[:], t_c2[:], t_d2[:])
                    t_p128 = big.tile([1, F], F32)
                    nc.vector.tensor_mul(t_p128[:], t_h128[:], p_n128[:])

            # ---------- impulse + Hs (gpsimd queues; overlaps stage B) ----------
            t_e1 = work.tile([1, 1], F32)
            nc.scalar.activation(t_e1[:], t_dec[:], AF.Exp, scale=-1.0)
            t_sp = work.tile([1, 1], F32)
            nc.scalar.activation(t_sp[:], t_e1[:], AF.Ln, bias=1.0, scale=1.0)
            t_s32 = work.tile([1, 1], F32)
            nc.vector.tensor_scalar(out=t_s32[:], in0=t_sp[:], scalar1=-1.0 / 32.0,
                                    scalar2=None, op0=OP.mult)
            t_ew = work.tile([1, 1], F32)
            nc.scalar.activation(t_ew[:], t_wet[:], AF.Exp, scale=-1.0)
            t_ew1 = work.tile([1, 1], F32)
            nc.vector.tensor_scalar(out=t_ew1[:], in0=t_ew[:], scalar1=1.0,
                                    scalar2=None, op0=OP.add)
            t_sw = work.tile([1, 1], F32)
            nc.vector.reciprocal(t_sw[:], t_ew1[:])
            t_s32b = work.tile([128, 1], F32)
            nc.gpsimd.partition_broadcast(t_s32b[:], t_s32[:])
            t_swb = work.tile([128, 1], F32)
            nc.gpsimd.partition_broadcast(t_swb[:], t_sw[:])
            t_env = work.tile([128, 125], F32)
            nc.scalar.activation(t_env[:], t_iotf[:], AF.Exp, scale=t_s32b[:, :])
            t_h = work.tile([128, 125], F32)
            nc.vector.scalar_tensor_tensor(out=t_h[:], in0=t_env[:], scalar=t_swb[:, :],
                                           in1=t_rn[:], op0=OP.mult, op1=OP.mult)
            nc.vector.memset(t_h[0:1, 0:1], 1.0)
            t_z = work.tile([1, 128], F32)
            nc.vector.memset(t_z[:], 0.0)
            nc.gpsimd.dma_start(bass.AP(hpbuf, 0, [[1, 1], [1, 127]]), t_z[0:1, 0:127])
            nc.gpsimd.dma_start(bass.AP(hpbuf, 127, [[1, 128], [128, 125]]), t_h[:])
            nc.gpsimd.dma_start(bass.AP(hpbuf, 127 + 16000, [[1, 1], [1, 128]]), t_z[0:1, 0:128])
            t_hs = big.tile([128, 16128], F32)
            for c in range(128):
                nc.gpsimd.dma_start(t_hs[c:c + 1, :],
                                    bass.AP(hpbuf, 127 - c, [[1, 1], [1, 16128]]))

            # ---------- harmonic stage B: depth-4 software pipeline ----------
            with tc.tile_pool(name="pskw", bufs=3, space="PSUM") as pskw, \
                 tc.tile_pool(name="psO", bufs=1, space="PSUM") as psO, \
                 tc.tile_pool(name="psnz2", bufs=1, space="PSUM") as psnz:
                p_nz = psnz.tile([128, F], F32)
                nc.tensor.matmul(p_nz[:], t_icre[:], t_pre[:], start=True, stop=False)
                nc.tensor.matmul(p_nz[:], t_icim[:], t_pim[:], start=False, stop=False)
                nc.tensor.matmul(p_nz[:], t_nyq[:], t_p128[:], start=False, stop=True)

                p_O = psO.tile([128, F], F32)
                kws, t1s, svs, sns = {}, {}, {}, {}
                for c in range(NCHUNK + 4):
                    if c < NCHUNK:
                        t_wc = chk.tile([1, CHW], F32, tag="wc")
                        nc.sync.dma_start(t_wc[:], bass.AP(wscr, c * CHW, [[1, 1], [1, CHW]]))
                        p_kw = pskw.tile([128, CHW], F32, tag="kw")
                        for half in range(2):
                            nc.tensor.matmul(p_kw[:, half * 512:(half + 1) * 512],
                                             t_kv[:], t_wc[0:1, half * 512:(half + 1) * 512],
                                             start=True, stop=True)
                        kws[c] = p_kw
                    if c - 1 >= 0 and c - 1 < NCHUNK:
                        pk = kws[c - 1]
                        t_t1 = chk.tile([128, CHW], F32, tag="t1")
                        nc.scalar.activation(t_t1[:], pk[:], AF.Identity,
                                             bias=t_mcol[:, :], scale=1.0)
                        t1s[c - 1] = t_t1
                    if c - 2 >= 0 and c - 2 < NCHUNK:
                        t_sv = chk.tile([128, CHW], F32, tag="sv")
                        nc.vector.scalar_tensor_tensor(out=t_sv[:], in0=t1s.pop(c - 2)[:],
                                                       scalar=MAGIC, in1=kws.pop(c - 2)[:],
                                                       op0=OP.subtract, op1=OP.subtract)
                        svs[c - 2] = t_sv
                    if c - 3 >= 0 and c - 3 < NCHUNK:
                        t_sn = chk.tile([128, CHW], F16, tag="sn")
                        nc.scalar.activation(t_sn[:], svs.pop(c - 3)[:], AF.Sin,
                                             scale=2.0 * math.pi)
                        sns[c - 3] = t_sn
                    if c - 4 >= 0:
                        cj = c - 4
                        t_sn = sns.pop(cj)
                        for fl in range(8):
                            f = 8 * cj + fl
                            nc.tensor.matmul(p_O[:, f:f + 1],
                                             t_sn[:, fl * 128:(fl + 1) * 128],
                                             t_at16[:, f:f + 1], start=True, stop=True)

                # ---------- combine ----------
                t_spad = big.tile([128, 526], F32)
                nc.vector.memset(t_spad[:, 0:126], 0.0)
                nc.scalar.copy(t_spad[:, 126:526], p_O[:])
                nc.vector.tensor_add(t_spad[:, 126:526], t_spad[:, 126:526], p_nz[:])

            # ---------- reverb ----------
            with tc.tile_pool(name="psrev", bufs=1, space="PSUM") as psr:
                p_rev = psr.tile([128, F], F32)
                for d in range(126):
                    nc.tensor.matmul(p_rev[:], t_hs[:, d * 128:d * 128 + 128],
                                     t_spad[:, 126 - d:526 - d],
                                     start=(d == 0), stop=(d == 125))
                t_out = big.tile([128, F], F32)
                nc.scalar.copy(t_out[:], p_rev[:])
                nc.sync.dma_start(out_d[:], t_out[:])

    nc.compile()
    return nc


def kernel(**inputs):
    if "nc" not in _cache:
        _cache["nc"] = _build()
        _cache["consts"] = _host_constants()
    nc = _cache["nc"]
    consts = _cache["consts"]

    amp = np.ascontiguousarray(np.asarray(inputs["amp_param"], np.float32))
    npr = np.ascontiguousarray(np.asarray(inputs["noise_param"], np.float32))
    pit = np.ascontiguousarray(np.asarray(inputs["pitch"], np.float32))
    noi = np.ascontiguousarray(np.asarray(inputs["noise"], np.float32))
    rvn = np.ascontiguousarray(np.asarray(inputs["reverb_noise"], np.float32))
    dec = np.asarray(inputs["decay"], np.float32).reshape(1, 1)
    wet = np.asarray(inputs["wet"], np.float32).reshape(1, 1)

    in_maps = []
    for b in range(B):
        m = dict(amp_param=amp[b], noise_param=npr[b], pitch=pit[b],
                 noise=noi[b], reverb_noise=rvn, decay=dec, wet=wet)
        m.update(consts)
        in_maps.append(m)

    res = run_bass_kernel_spmd(nc, in_maps, list(range(B)))
    out = np.stack([res.results[b]["out"].T.reshape(T, 1) for b in range(B)])
    return out.astype(np.float32)


if __name__ == "__main__":
    rng = np.random.default_rng(0)
    ins = dict(
        amp_param=rng.standard_normal((B, F, NH + 1)).astype(np.float32),
        noise_param=rng.standard_normal((B, F, NB)).astype(np.float32),
        pitch=(rng.random((B, F, 1), np.float32) * 440 + 60),
        noise=(rng.random((B, F, BLOCK), np.float32) * 2 - 1),
        reverb_noise=(rng.random((SR, 1), np.float32) * 2 - 1),
        decay=np.ones(1, np.float32) * 5,
        wet=np.zeros(1, np.float32),
        sampling_rate=SR, block_size=BLOCK,
    )
    o = kernel(**ins)
    print("kernel out", o.shape, o.dtype, np.abs(o).max())


def _install_ntff_hook():
    import sys as _sys
    import types as _types
    try:
        import antenv.axon_hooks  # noqa: F401
        return
    except ImportError:
        pass
    from trn_agent_boot.trn_boot import _ntff_profile_via_ctypes
    hook = _ntff_profile_via_ctypes('/opt/axon/libaxon_pjrt.so')
    mod = _types.ModuleType('antenv.axon_hooks')
    _h = {'v': hook}
    mod.get_axon_ntff_profile_hook = lambda: _h['v']
    mod.set_axon_ntff_profile_hook = lambda h: _h.update(v=h)
    _sys.modules['antenv.axon_hooks'] = mod
    import antenv
    antenv.axon_hooks = mod


def run_timed(**inputs):
    """Re-run with NTFF tracing enabled; returns max per-core exec ns or None."""
    _install_ntff_hook()
    if "nc" not in _cache:
        _cache["nc"] = _build()
        _cache["consts"] = _host_constants()
    nc = _cache["nc"]
    consts = _cache["consts"]
    amp = np.ascontiguousarray(np.asarray(inputs["amp_param"], np.float32))
    npr = np.ascontiguousarray(np.asarray(inputs["noise_param"], np.float32))
    pit = np.ascontiguousarray(np.asarray(inputs["pitch"], np.float32))
    noi = np.ascontiguousarray(np.asarray(inputs["noise"], np.float32))
    rvn = np.ascontiguousarray(np.asarray(inputs["reverb_noise"], np.float32))
    dec = np.asarray(inputs["decay"], np.float32).reshape(1, 1)
    wet = np.asarray(inputs["wet"], np.float32).reshape(1, 1)
    in_maps = []
    for b in range(B):
        m = dict(amp_param=amp[b], noise_param=npr[b], pitch=pit[b],
                 noise=noi[b], reverb_noise=rvn, decay=dec, wet=wet)
        m.update(consts)
        in_maps.append(m)
    res = run_bass_kernel_spmd(nc, in_maps, list(range(B)), trace=True)
    if res.instructions_and_trace is not None:
        _cache["insts"] = res.instructions_and_trace[0]
    return res.exec_time_ns



# revision 2
# speedup vs baseline: 2.6827x; 2.6827x over previous
# DDSP synthesizer kernel for Trainium2 (8 NeuronCores, batch-parallel).
#
# Per core (one batch element):
#   harmonic branch: exact-phase oscillator bank. Phases are wrapped to
#     [-0.5,0.5] in (j,f) layout, split hi/lo bf16, PE-transposed to
#     time-major, then k*w computed as a K=2 bf16 outer-product matmul
#     (exact products, fp32 PSUM accumulate). Magic-number wrap
#     (ScalarE/DVE split), ScalarE Sin, per-frame weighted reduction on PE.
#   noise branch: irfft+window folded into one DFT matmul, frame-wise
#     128-tap causal conv via rfft-256 as PE matmuls (inverse side bf16).
#   reverb: 16000-tap causal FIR as 126 accumulating bf16 Toeplitz matmuls;
#     the Toeplitz operand is fetched with 4 wide positive-stride DMAs
#     (samples are generated block-reversed so the shift structure needs
#     no negative strides).
import math
import numpy as np

import concourse.bass as bass
import concourse.bacc as bacc
import concourse.mybir as mybir
from concourse import tile
from concourse.bass_utils import run_bass_kernel_spmd

F32 = mybir.dt.float32
F16 = mybir.dt.float16
BF16 = mybir.dt.bfloat16
B, F, NH, NB = 8, 400, 100, 65
SR, BLOCK = 16000, 128
T = F * BLOCK
LOG10 = math.log(10.0)
MAGIC = 12582912.0  # 1.5 * 2**23
NCHUNK = 50         # harmonic chunks of 1024 samples
CHW = 1024

_cache = {}


def _host_constants():
    b = np.arange(NB)[:, None]
    m = np.arange(128)[None, :]
    w = np.where((b == 0) | (b == 64), 1.0, 2.0)
    Cmat = w / 128.0 * np.cos(2 * np.pi * b * m / 128.0)
    win2 = 0.5 + 0.5 * np.cos(2 * np.pi * np.arange(128) / 128.0)
    Air = (2.0 * Cmat * win2[None, :]).astype(np.float32)               # (65,128)
    ccol = (1e-7 * (Cmat * win2[None, :]).sum(0)).astype(np.float32)[:, None]  # (128,1)
    j = np.arange(128)[:, None]
    bb = np.arange(128)[None, :]
    Dcos = np.cos(2 * np.pi * j * bb / 256.0).astype(np.float32)        # (128j,128b)
    Dsin = (-np.sin(2 * np.pi * j * bb / 256.0)).astype(np.float32)
    d128 = np.cos(np.pi * np.arange(128)).astype(np.float32)[:, None]   # (128,1)
    bb2 = np.arange(128)[:, None]
    i = np.arange(128)[None, :]
    cb = np.where(bb2 == 0, 1.0, 2.0)
    ICre = (cb / 256.0 * np.cos(2 * np.pi * bb2 * i / 256.0)).astype(np.float32)
    ICim = (-2.0 / 256.0 * np.sin(2 * np.pi * bb2 * i / 256.0)).astype(np.float32)
    nyq = ((1.0 / 256.0) * np.cos(np.pi * np.arange(128))).astype(np.float32)[None, :]
    # output samples are produced block-reversed (partition p = sample 127-p)
    ICre = np.ascontiguousarray(ICre[:, ::-1])
    ICim = np.ascontiguousarray(ICim[:, ::-1])
    nyq = np.ascontiguousarray(nyq[:, ::-1])
    kvneg = np.zeros((1, 128), np.float32)
    kvneg[0, :NH] = -np.arange(1, NH + 1)
    eye = np.eye(128, dtype=np.float32)
    return dict(c_air=Air, c_ccol=ccol, c_dcos=Dcos, c_dsin=Dsin, c_d128=d128,
                c_icre=ICre, c_icim=ICim, c_nyq=nyq, c_kvneg=kvneg, c_eye=eye)


def _build():
    nc = bacc.Bacc(None, target_bir_lowering=False, debug=False)

    amp_d = nc.dram_tensor("amp_param", [F, NH + 1], F32, kind="ExternalInput")
    npr_d = nc.dram_tensor("noise_param", [F, NB], F32, kind="ExternalInput")
    pit_d = nc.dram_tensor("pitch", [F, 1], F32, kind="ExternalInput")
    noi_d = nc.dram_tensor("noise", [F, BLOCK], F32, kind="ExternalInput")
    rvn_d = nc.dram_tensor("reverb_noise", [SR, 1], F32, kind="ExternalInput")
    dec_d = nc.dram_tensor("decay", [1, 1], F32, kind="ExternalInput")
    wet_d = nc.dram_tensor("wet", [1, 1], F32, kind="ExternalInput")
    cst = {}
    for name, shape in [("c_air", [NB, 128]), ("c_ccol", [128, 1]),
                        ("c_dcos", [128, 128]), ("c_dsin", [128, 128]),
                        ("c_d128", [128, 1]), ("c_icre", [128, 128]),
                        ("c_icim", [128, 128]), ("c_nyq", [1, 128]),
                        ("c_kvneg", [1, 128]), ("c_eye", [128, 128])]:
        cst[name] = nc.dram_tensor(name, shape, F32, kind="ExternalInput")
    out_d = nc.dram_tensor("out", [128, F], F32, kind="ExternalOutput")

    wscr16 = nc.dram_tensor("wscr16", [2, T], BF16)
    hpbuf16 = nc.dram_tensor("hpbuf16", [1, 16384], BF16)

    AF = mybir.ActivationFunctionType
    OP = mybir.AluOpType

    with tile.TileContext(nc) as tc:
        with tc.tile_pool(name="const", bufs=1) as cpool, \
             tc.tile_pool(name="big", bufs=1) as big, \
             tc.tile_pool(name="work", bufs=1) as work, \
             tc.tile_pool(name="chunk", bufs=3) as chk:

            # ---------- constants ----------
            t_mcol = cpool.tile([128, 1], F32)
            nc.vector.memset(t_mcol[:], MAGIC)
            t_eps = cpool.tile([128, 1], F32)
            nc.vector.memset(t_eps[:], 1e-7)
            t_b5 = cpool.tile([128, 1], F32)
            nc.vector.memset(t_b5[:], 5.0)
            t_kv2f = cpool.tile([2, 128], F32)
            nc.sync.dma_start(t_kv2f[:], bass.AP(cst["c_kvneg"], 0, [[0, 2], [1, 128]]))
            t_kv16 = cpool.tile([2, 128], BF16)
            nc.vector.tensor_copy(t_kv16[:], t_kv2f[:])
            t_air = cpool.tile([NB, 128], F32)
            nc.sync.dma_start(t_air[:], cst["c_air"][:])
            t_ccol = cpool.tile([128, 1], F32)
            nc.sync.dma_start(t_ccol[:], cst["c_ccol"][:])
            t_dcos = cpool.tile([128, 128], F32)
            nc.sync.dma_start(t_dcos[:], cst["c_dcos"][:])
            t_dsin = cpool.tile([128, 128], F32)
            nc.sync.dma_start(t_dsin[:], cst["c_dsin"][:])
            t_d128 = cpool.tile([128, 1], F32)
            nc.sync.dma_start(t_d128[:], cst["c_d128"][:])
            t_icre32 = cpool.tile([128, 128], F32)
            nc.sync.dma_start(t_icre32[:], cst["c_icre"][:])
            t_icre16 = cpool.tile([128, 128], BF16)
            nc.vector.tensor_copy(t_icre16[:], t_icre32[:])
            t_icim32 = cpool.tile([128, 128], F32)
            nc.sync.dma_start(t_icim32[:], cst["c_icim"][:])
            t_icim16 = cpool.tile([128, 128], BF16)
            nc.vector.tensor_copy(t_icim16[:], t_icim32[:])
            t_nyq32 = cpool.tile([1, 128], F32)
            nc.sync.dma_start(t_nyq32[:], cst["c_nyq"][:])
            t_nyq16 = cpool.tile([1, 128], BF16)
            nc.vector.tensor_copy(t_nyq16[:], t_nyq32[:])
            t_eye = cpool.tile([128, 128], F32)
            nc.sync.dma_start(t_eye[:], cst["c_eye"][:])
            t_eye16 = cpool.tile([128, 128], BF16)
            nc.vector.tensor_copy(t_eye16[:], t_eye[:])
            t_kroi = cpool.tile([128, NH], mybir.dt.int32)
            nc.gpsimd.iota(t_kroi[:], pattern=[[1, NH]], base=1, channel_multiplier=0)
            t_krow = cpool.tile([128, NH], F32)
            nc.vector.tensor_copy(t_krow[:], t_kroi[:])
            t_jp1i = cpool.tile([128, 1], mybir.dt.int32)
            nc.gpsimd.iota(t_jp1i[:], pattern=[[1, 1]], base=1, channel_multiplier=1)
            t_jp1 = cpool.tile([128, 1], F32)
            nc.vector.tensor_copy(t_jp1[:], t_jp1i[:])
            t_jrev = cpool.tile([128, 1], F32)
            nc.vector.tensor_scalar(out=t_jrev[:], in0=t_jp1[:], scalar1=-1.0,
                                    scalar2=129.0, op0=OP.mult, op1=OP.add)
            t_ioti = cpool.tile([128, 125], mybir.dt.int32)
            nc.gpsimd.iota(t_ioti[:], pattern=[[128, 125]], base=0, channel_multiplier=1)
            t_iotf = cpool.tile([128, 125], F32)
            nc.vector.tensor_copy(t_iotf[:], t_ioti[:])

            # early loads for impulse build
            t_dec = work.tile([1, 1], F32)
            nc.sync.dma_start(t_dec[:], dec_d[:])
            t_wet = work.tile([1, 1], F32)
            nc.sync.dma_start(t_wet[:], wet_d[:])
            t_rn = work.tile([128, 125], F32)
            nc.sync.dma_start(t_rn[:], bass.AP(rvn_d, 0, [[1, 128], [128, 125]]))

            with tc.high_priority():
                # ---------- phase chain (critical path to stage B) ----------
                t_pr = work.tile([1, F], F32)
                nc.sync.dma_start(t_pr[:], bass.AP(pit_d, 0, [[1, 1], [1, F]]))
                t_p8 = work.tile([1, F], F32)
                nc.scalar.activation(t_p8[:], t_pr[:], AF.Identity,
                                     bias=t_mcol[0:1, :], scale=8.0)
                t_ph = work.tile([1, F], F32)
                nc.vector.tensor_scalar(out=t_ph[:], in0=t_p8[:], scalar1=MAGIC,
                                        scalar2=0.125, op0=OP.subtract, op1=OP.mult)
                t_pl = work.tile([1, F], F32)
                nc.vector.tensor_sub(t_pl[:], t_pr[:], t_ph[:])
                t_zr = work.tile([1, F], F32)
                nc.vector.memset(t_zr[:], 0.0)
                t_sh = work.tile([1, F + 1], F32)
                nc.vector.memset(t_sh[:, 0:1], 0.0)
                nc.vector.tensor_tensor_scan(t_sh[:, 1:F + 1], t_ph[:], t_zr[:], 0.0,
                                             OP.add, OP.add)
                t_sl = work.tile([1, F + 1], F32)
                nc.vector.memset(t_sl[:, 0:1], 0.0)
                nc.vector.tensor_tensor_scan(t_sl[:, 1:F + 1], t_pl[:], t_zr[:], 0.0,
                                             OP.add, OP.add)
                t_ts = work.tile([1, F], F32)
                nc.vector.tensor_add(t_ts[:], t_sh[:, 0:F], t_sl[:, 0:F])
                t_t2r = work.tile([1, F], F32)
                nc.scalar.activation(t_t2r[:], t_ts[:], AF.Identity,
                                     bias=t_mcol[0:1, :], scale=1.0 / 125.0)
                t_n125 = work.tile([1, F], F32)
                nc.vector.tensor_scalar(out=t_n125[:], in0=t_t2r[:], scalar1=MAGIC,
                                        scalar2=None, op0=OP.subtract)
                t_u = work.tile([1, F], F32)
                nc.vector.scalar_tensor_tensor(out=t_u[:], in0=t_n125[:], scalar=-125.0,
                                               in1=t_sh[:, 0:F], op0=OP.mult, op1=OP.add)
                t_r125 = work.tile([1, F], F32)
                nc.vector.tensor_add(t_r125[:], t_u[:], t_sl[:, 0:F])
                t_om = work.tile([1, F], F32)
                nc.vector.tensor_scalar(out=t_om[:], in0=t_r125[:], scalar1=1.0 / 125.0,
                                        scalar2=None, op0=OP.mult)
                t_cr = work.tile([1, F], F32)
                nc.vector.tensor_scalar(out=t_cr[:], in0=t_pr[:], scalar1=1.0 / 16000.0,
                                        scalar2=None, op0=OP.mult)
                t_omb = work.tile([128, F], F32)
                nc.gpsimd.partition_broadcast(t_omb[:], t_om[:])
                t_cb = work.tile([128, F], F32)
                nc.gpsimd.partition_broadcast(t_cb[:], t_cr[:])
                t_wraw = work.tile([128, F], F32)
                nc.vector.scalar_tensor_tensor(out=t_wraw[:], in0=t_cb[:],
                                               scalar=t_jrev[:, :], in1=t_omb[:],
                                               op0=OP.mult, op1=OP.add)
                t_tw = work.tile([128, F], F32)
                nc.scalar.activation(t_tw[:], t_wraw[:], AF.Identity,
                                     bias=t_mcol[:, :], scale=1.0)
                t_wneg = work.tile([128, F], F32)
                nc.vector.scalar_tensor_tensor(out=t_wneg[:], in0=t_tw[:], scalar=MAGIC,
                                               in1=t_wraw[:], op0=OP.subtract,
                                               op1=OP.subtract)
                t_whi16 = work.tile([128, F], BF16)
                nc.vector.tensor_copy(t_whi16[:], t_wneg[:])
                t_wlo16 = work.tile([128, F], BF16)
                nc.vector.tensor_sub(t_wlo16[:], t_wneg[:], t_whi16[:])
                with tc.tile_pool(name="pswt", bufs=2, space="PSUM") as pswt:
                    t_wThi = work.tile([100, 512], BF16)
                    t_wTlo = work.tile([100, 512], BF16)
                    for g in range(4):
                        p_th = pswt.tile([100, 128], BF16, tag="wt")
                        nc.tensor.transpose(p_th[:], t_whi16[:, g * 100:(g + 1) * 100],
                                            t_eye16[:])
                        nc.scalar.copy(t_wThi[:, g * 128:(g + 1) * 128], p_th[:])
                        p_tl = pswt.tile([100, 128], BF16, tag="wt")
                        nc.tensor.transpose(p_tl[:], t_wlo16[:, g * 100:(g + 1) * 100],
                                            t_eye16[:])
                        nc.scalar.copy(t_wTlo[:, g * 128:(g + 1) * 128], p_tl[:])
                    for g in range(4):
                        nc.sync.dma_start(
                            bass.AP(wscr16, g * 12800, [[128, 100], [1, 128]]),
                            t_wThi[:, g * 128:(g + 1) * 128])
                        nc.sync.dma_start(
                            bass.AP(wscr16, T + g * 12800, [[128, 100], [1, 128]]),
                            t_wTlo[:, g * 128:(g + 1) * 128])

            # ---------- impulse + Toeplitz gather (gpsimd queue) ----------
            t_e1 = work.tile([1, 1], F32)
            nc.scalar.activation(t_e1[:], t_dec[:], AF.Exp, scale=-1.0)
            t_sp = work.tile([1, 1], F32)
            nc.scalar.activation(t_sp[:], t_e1[:], AF.Ln, bias=1.0, scale=1.0)
            t_s32 = work.tile([1, 1], F32)
            nc.vector.tensor_scalar(out=t_s32[:], in0=t_sp[:], scalar1=-1.0 / 32.0,
                                    scalar2=None, op0=OP.mult)
            t_ew = work.tile([1, 1], F32)
            nc.scalar.activation(t_ew[:], t_wet[:], AF.Exp, scale=-1.0)
            t_ew1 = work.tile([1, 1], F32)
            nc.vector.tensor_scalar(out=t_ew1[:], in0=t_ew[:], scalar1=1.0,
                                    scalar2=None, op0=OP.add)
            t_sw = work.tile([1, 1], F32)
            nc.vector.reciprocal(t_sw[:], t_ew1[:])
            t_s32b = work.tile([128, 1], F32)
            nc.gpsimd.partition_broadcast(t_s32b[:], t_s32[:])
            t_swb = work.tile([128, 1], F32)
            nc.gpsimd.partition_broadcast(t_swb[:], t_sw[:])
            t_env = work.tile([128, 125], F32)
            nc.scalar.activation(t_env[:], t_iotf[:], AF.Exp, scale=t_s32b[:, :])
            t_h = work.tile([128, 125], F32)
            nc.vector.scalar_tensor_tensor(out=t_h[:], in0=t_env[:], scalar=t_swb[:, :],
                                           in1=t_rn[:], op0=OP.mult, op1=OP.mult)
            nc.vector.memset(t_h[0:1, 0:1], 1.0)
            t_h16 = work.tile([128, 125], BF16)
            nc.vector.tensor_copy(t_h16[:], t_h[:])
            t_z16 = work.tile([1, 160], BF16)
            nc.vector.memset(t_z16[:], 0.0)
            nc.gpsimd.dma_start(bass.AP(hpbuf16, 0, [[1, 1], [1, 127]]),
                                t_z16[0:1, 0:127])
            nc.gpsimd.dma_start(bass.AP(hpbuf16, 127, [[1, 128], [128, 125]]), t_h16[:])
            nc.gpsimd.dma_start(bass.AP(hpbuf16, 16127, [[1, 1], [1, 129]]),
                                t_z16[0:1, 0:129])
            t_hs16 = big.tile([128, 16128], BF16)
            for q in range(4):
                nc.gpsimd.dma_start(
                    t_hs16[q * 32:(q + 1) * 32, :],
                    bass.AP(hpbuf16, q * 32, [[1, 32], [128, 126], [1, 128]]))

            with tc.tile_pool(name="pstr", bufs=2, space="PSUM") as pstr:
                # ---------- amp params ----------
                t_at = big.tile([128, F], F32)
                nc.vector.memset(t_at[:], 0.0)
                amp_sf = []
                for g in range(4):
                    f0 = g * 100
                    t_ap = work.tile([100, NH + 1], F32, tag=f"ampchunk{g}", name=f"ampchunk{g}")
                    nc.sync.dma_start(t_ap[:], amp_d[f0:f0 + 100, :])
                    t_sf = work.tile([100, NH + 1], F32, tag=f"ampsf{g}", name=f"ampsf{g}")
                    nc.scalar.activation(t_sf[:], t_ap[:], AF.Exp, scale=-1.0)
                    amp_sf.append(t_sf)
                ns_sf = []
                for g in range(4):
                    f0 = g * 100
                    t_np = work.tile([100, NB], F32, tag=f"npchunk{g}", name=f"npchunk{g}")
                    nc.sync.dma_start(t_np[:], npr_d[f0:f0 + 100, :])
                    t_ns = work.tile([100, NB], F32, tag=f"nsf{g}", name=f"nsf{g}")
                    nc.scalar.activation(t_ns[:], t_np[:], AF.Exp, bias=t_b5[0:100, :], scale=-1.0)
                    ns_sf.append(t_ns)
                for t_sf in amp_sf + ns_sf:
                    nc.scalar.activation(t_sf[:], t_sf[:], AF.Ln, bias=1.0, scale=1.0)
                for t_sf in amp_sf + ns_sf:
                    nc.scalar.activation(t_sf[:], t_sf[:], AF.Exp, scale=-LOG10)
                for t_sf in amp_sf:
                    nc.scalar.activation(t_sf[:], t_sf[:], AF.Identity,
                                         bias=t_eps[0:100, :], scale=2.0)
                t_pcol = work.tile([100, 4], F32)
                nc.sync.dma_start(t_pcol[:], bass.AP(pit_d, 0, [[1, 100], [100, 4]]))
                for g in range(4):
                    f0 = g * 100
                    t_sf = amp_sf[g]
                    t_kp = work.tile([100, NH], F32, tag="kp")
                    nc.vector.tensor_scalar(out=t_kp[:], in0=t_krow[0:100, :],
                                            scalar1=t_pcol[:, g:g + 1], scalar2=None, op0=OP.mult)
                    t_aa = work.tile([100, NH], F32, tag="aa")
                    nc.vector.tensor_scalar(out=t_aa[:], in0=t_kp[:], scalar1=8000.0,
                                            scalar2=1e-4, op0=OP.is_lt, op1=OP.add)
                    t_am = work.tile([100, NH], F32, tag="am")
                    nc.vector.tensor_mul(t_am[:], t_sf[:, 1:NH + 1], t_aa[:])
                    t_ssum = work.tile([100, 1], F32, tag="ssum")
                    nc.vector.tensor_reduce(out=t_ssum[:], in_=t_am[:],
                                            axis=mybir.AxisListType.X, op=OP.add)
                    t_rec = work.tile([100, 1], F32, tag="rec")
                    nc.vector.reciprocal(t_rec[:], t_ssum[:])
                    t_scn = work.tile([100, 1], F32, tag="scn")
                    nc.vector.tensor_mul(t_scn[:], t_rec[:], t_sf[:, 0:1])
                    nc.vector.tensor_scalar(out=t_scn[:], in0=t_scn[:], scalar1=-1.0,
                                            scalar2=None, op0=OP.mult)
                    nc.vector.tensor_scalar(out=t_am[:], in0=t_am[:], scalar1=t_scn[:, :],
                                            scalar2=None, op0=OP.mult)
                    p_tr = pstr.tile([100, 100], F32, tag="tr")
                    nc.tensor.transpose(p_tr[:], t_am[:], t_eye[0:100, 0:100])
                    nc.scalar.copy(t_at[0:100, f0:f0 + 100], p_tr[:])
                t_at16 = big.tile([128, F], F16)
                nc.vector.tensor_copy(t_at16[:], t_at[:])

                # ---------- noise branch ----------
                t_nt = big.tile([65, F], F32)
                for g in range(4):
                    f0 = g * 100
                    p_tr2 = pstr.tile([65, 100], F32, tag="tr")
                    nc.tensor.transpose(p_tr2[:], ns_sf[g][:], t_eye[0:100, 0:100])
                    nc.scalar.copy(t_nt[0:65, f0:f0 + 100], p_tr2[:])
                t_noT = big.tile([128, F], F32)
                nc.sync.dma_start(t_noT[:], bass.AP(noi_d, 0, [[1, 128], [128, F]]))

                with tc.tile_pool(name="psn", bufs=1, space="PSUM") as psn:
                    p_irp = psn.tile([128, F], F32, tag="tmp", bufs=2)
                    nc.tensor.matmul(p_irp[:], t_air[:], t_nt[0:65, :], start=True, stop=True)
                    t_irp = big.tile([128, F], F32)
                    nc.vector.tensor_scalar(out=t_irp[:], in0=p_irp[:],
                                            scalar1=t_ccol[:, :], scalar2=None, op0=OP.add)
                    p_hre = psn.tile([128, F], F32, tag="tmp", bufs=2)
                    nc.tensor.matmul(p_hre[:], t_dcos[:], t_irp[:], start=True, stop=True)
                    t_hre = big.tile([128, F], F32)
                    nc.scalar.copy(t_hre[:], p_hre[:])
                    p_him = psn.tile([128, F], F32, tag="tmp", bufs=2)
                    nc.tensor.matmul(p_him[:], t_dsin[:], t_irp[:], start=True, stop=True)
                    t_him = big.tile([128, F], F32)
                    nc.scalar.copy(t_him[:], p_him[:])
                    p_h128 = psn.tile([1, F], F32, tag="tmp", bufs=2)
                    nc.tensor.matmul(p_h128[:], t_d128[:], t_irp[:], start=True, stop=True)
                    t_h128 = big.tile([1, F], F32)
                    nc.scalar.copy(t_h128[:], p_h128[:])
                    p_nre = psn.tile([128, F], F32, tag="nre")
                    nc.tensor.matmul(p_nre[:], t_dcos[:], t_noT[:], start=True, stop=True)
                    p_nim = psn.tile([128, F], F32, tag="nim")
                    nc.tensor.matmul(p_nim[:], t_dsin[:], t_noT[:], start=True, stop=True)
                    p_n128 = psn.tile([1, F], F32, tag="tmp", bufs=2)
                    nc.tensor.matmul(p_n128[:], t_d128[:], t_noT[:], start=True, stop=True)
                    t_a = work.tile([128, F], F32, tag="pa")
                    nc.vector.tensor_mul(t_a[:], t_hre[:], p_nre[:])
                    t_b = work.tile([128, F], F32, tag="pb")
                    nc.vector.tensor_mul(t_b[:], t_him[:], p_nim[:])
                    t_pre = big.tile([128, F], BF16)
                    nc.vector.tensor_sub(t_pre[:], t_a[:], t_b[:])
                    t_c2 = work.tile([128, F], F32, tag="pc2")
                    nc.vector.tensor_mul(t_c2[:], t_him[:], p_nre[:])
                    t_d2 = work.tile([128, F], F32, tag="pd2")
                    nc.vector.tensor_mul(t_d2[:], t_hre[:], p_nim[:])
                    t_pim = big.tile([128, F], BF16)
                    nc.vector.tensor_add(t_pim[:], t_c2[:], t_d2[:])
                    t_p128 = big.tile([1, F], BF16)
                    nc.vector.tensor_mul(t_p128[:], t_h128[:], p_n128[:])

            # ---------- harmonic stage B: depth-4 software pipeline ----------
            with tc.tile_pool(name="pskw", bufs=3, space="PSUM") as pskw, \
                 tc.tile_pool(name="psO", bufs=1, space="PSUM") as psO, \
                 tc.tile_pool(name="psnz2", bufs=1, space="PSUM") as psnz:
                p_nz = psnz.tile([128, F], F32)
                nc.tensor.matmul(p_nz[:], t_icre16[:], t_pre[:], start=True, stop=False)
                nc.tensor.matmul(p_nz[:], t_icim16[:], t_pim[:], start=False, stop=False)
                nc.tensor.matmul(p_nz[:], t_nyq16[:], t_p128[:], start=False, stop=True)

                p_O = psO.tile([128, F], F32)
                kws, t1s, svs, sns = {}, {}, {}, {}
                for c in range(NCHUNK + 4):
                    if c < NCHUNK:
                        t_wc = chk.tile([2, CHW], BF16, tag="wc")
                        nc.sync.dma_start(t_wc[:],
                                          bass.AP(wscr16, c * CHW, [[T, 2], [1, CHW]]))
                        p_kw = pskw.tile([128, CHW], F32, tag="kw")
                        for half in range(2):
                            nc.tensor.matmul(p_kw[:, half * 512:(half + 1) * 512],
                                             t_kv16[:], t_wc[:, half * 512:(half + 1) * 512],
                                             start=True, stop=True)
                        kws[c] = p_kw
                    if c - 1 >= 0 and c - 1 < NCHUNK:
                        cj = c - 1
                        pk = kws[cj]
                        t_t1 = chk.tile([128, CHW], F32, tag="t1")
                        if cj % 5 in (1, 3):
                            nc.vector.tensor_scalar(out=t_t1[:], in0=pk[:], scalar1=MAGIC,
                                                    scalar2=None, op0=OP.add)
                        else:
                            nc.scalar.activation(t_t1[:], pk[:], AF.Identity,
                                                 bias=t_mcol[:, :], scale=1.0)
                        t1s[cj] = t_t1
                    if c - 2 >= 0 and c - 2 < NCHUNK:
                        t_sv = chk.tile([128, CHW], F32, tag="sv")
                        nc.vector.scalar_tensor_tensor(out=t_sv[:], in0=t1s.pop(c - 2)[:],
                                                       scalar=MAGIC, in1=kws.pop(c - 2)[:],
                                                       op0=OP.subtract, op1=OP.subtract)
                        svs[c - 2] = t_sv
                    if c - 3 >= 0 and c - 3 < NCHUNK:
                        t_sn = chk.tile([128, CHW], F16, tag="sn")
                        nc.scalar.activation(t_sn[:], svs.pop(c - 3)[:], AF.Sin,
                                             scale=2.0 * math.pi)
                        sns[c - 3] = t_sn
                    if c - 4 >= 0:
                        cj = c - 4
                        t_sn = sns.pop(cj)
                        for fl in range(8):
                            f = 8 * cj + fl
                            nc.tensor.matmul(p_O[:, f:f + 1],
                                             t_sn[:, fl * 128:(fl + 1) * 128],
                                             t_at16[:, f:f + 1], start=True, stop=True)

                # ---------- combine (block-reversed samples) ----------
                t_spad16 = big.tile([128, 526], BF16)
                nc.vector.memset(t_spad16[:, 0:126], 0.0)
                nc.scalar.copy(t_spad16[:, 126:526], p_O[:])
                nc.vector.tensor_add(t_spad16[:, 126:526], t_spad16[:, 126:526], p_nz[:])

            # ---------- reverb: 126 accumulating bf16 Toeplitz matmuls ----------
            with tc.tile_pool(name="psrev", bufs=1, space="PSUM") as psr:
                p_rev = psr.tile([128, F], F32)
                for d in range(126):
                    nc.tensor.matmul(p_rev[:], t_hs16[:, d * 128:d * 128 + 128],
                                     t_spad16[:, 126 - d:526 - d],
                                     start=(d == 0), stop=(d == 125))
                t_out = big.tile([128, F], F32)
                nc.scalar.copy(t_out[:], p_rev[:])
                nc.sync.dma_start(out_d[:], t_out[:])

    nc.compile()
    return nc


def kernel(**inputs):
    if "nc" not in _cache:
        _cache["nc"] = _build()
        _cache["consts"] = _host_constants()
    nc = _cache["nc"]
    consts = _cache["consts"]

    amp = np.ascontiguousarray(np.asarray(inputs["amp_param"], np.float32))
    npr = np.ascontiguousarray(np.asarray(inputs["noise_param"], np.float32))
    pit = np.ascontiguousarray(np.asarray(inputs["pitch"], np.float32))
    noi = np.ascontiguousarray(np.asarray(inputs["noise"], np.float32))
    rvn = np.ascontiguousarray(np.asarray(inputs["reverb_noise"], np.float32))
    dec = np.asarray(inputs["decay"], np.float32).reshape(1, 1)
    wet = np.asarray(inputs["wet"], np.float32).reshape(1, 1)

    in_maps = []
    for b in range(B):
        m = dict(amp_param=amp[b], noise_param=npr[b], pitch=pit[b],
                 noise=noi[b], reverb_noise=rvn, decay=dec, wet=wet)
        m.update(consts)
        in_maps.append(m)

    res = run_bass_kernel_spmd(nc, in_maps, list(range(B)))
    out = np.stack([res.results[b]["out"].T.reshape(T, 1) for b in range(B)])
    return out.astype(np.float32)


if __name__ == "__main__":
    rng = np.random.default_rng(0)
    ins = dict(
        amp_param=rng.standard_normal((B, F, NH + 1)).astype(np.float32),
        noise_param=rng.standard_normal((B, F, NB)).astype(np.float32),
        pitch=(rng.random((B, F, 1), np.float32) * 440 + 60),
        noise=(rng.random((B, F, BLOCK), np.float32) * 2 - 1),
        reverb_noise=(rng.random((SR, 1), np.float32) * 2 - 1),
        decay=np.ones(1, np.float32) * 5,
        wet=np.zeros(1, np.float32),
        sampling_rate=SR, block_size=BLOCK,
    )
    o = kernel(**ins)
    print("kernel out", o.shape, o.dtype, np.abs(o).max())


def _install_ntff_hook():
    import sys as _sys
    import types as _types
    try:
        import antenv.axon_hooks  # noqa: F401
        return
    except ImportError:
        pass
    from trn_agent_boot.trn_boot import _ntff_profile_via_ctypes
    hook = _ntff_profile_via_ctypes('/opt/axon/libaxon_pjrt.so')
    mod = _types.ModuleType('antenv.axon_hooks')
    _h = {'v': hook}
    mod.get_axon_ntff_profile_hook = lambda: _h['v']
    mod.set_axon_ntff_profile_hook = lambda h: _h.update(v=h)
    _sys.modules['antenv.axon_hooks'] = mod
    import antenv
    antenv.axon_hooks = mod


def run_timed(**inputs):
    """Re-run with NTFF tracing enabled; returns max per-core exec ns or None."""
    _install_ntff_hook()
    if "nc" not in _cache:
        _cache["nc"] = _build()
        _cache["consts"] = _host_constants()
    nc = _cache["nc"]
    consts = _cache["consts"]
    amp = np.ascontiguousarray(np.asarray(inputs["amp_param"], np.float32))
    npr = np.ascontiguousarray(np.asarray(inputs["noise_param"], np.float32))
    pit = np.ascontiguousarray(np.asarray(inputs["pitch"], np.float32))
    noi = np.ascontiguousarray(np.asarray(inputs["noise"], np.float32))
    rvn = np.ascontiguousarray(np.asarray(inputs["reverb_noise"], np.float32))
    dec = np.asarray(inputs["decay"], np.float32).reshape(1, 1)
    wet = np.asarray(inputs["wet"], np.float32).reshape(1, 1)
    in_maps = []
    for b in range(B):
        m = dict(amp_param=amp[b], noise_param=npr[b], pitch=pit[b],
                 noise=noi[b], reverb_noise=rvn, decay=dec, wet=wet)
        m.update(consts)
        in_maps.append(m)
    res = run_bass_kernel_spmd(nc, in_maps, list(range(B)), trace=True)
    if res.instructions_and_trace is not None:
        _cache["insts"] = res.instructions_and_trace[0]
    return res.exec_time_ns


# revision 5
# speedup vs baseline: 2.8323x; 1.0558x over previous
# DDSP synthesizer kernel for Trainium2 (8 NeuronCores, batch-parallel).
#
# Per core (one batch element):
#   harmonic branch: exact-phase oscillator bank. Phases are wrapped to
#     [-0.5,0.5] in (j,f) layout, split hi/lo bf16, PE-transposed to
#     time-major, then k*w computed as a K=2 bf16 outer-product matmul
#     (exact products, fp32 PSUM accumulate). Magic-number wrap
#     (ScalarE/DVE split), ScalarE Sin, per-frame weighted reduction on PE.
#   noise branch: irfft+window folded into one DFT matmul, frame-wise
#     128-tap causal conv via rfft-256 as PE matmuls (inverse side bf16).
#   reverb: 16000-tap causal FIR as 126 accumulating bf16 Toeplitz matmuls;
#     the Toeplitz operand is fetched with 4 wide positive-stride DMAs
#     (samples are generated block-reversed so the shift structure needs
#     no negative strides).
import math
import numpy as np

import concourse.bass as bass
import concourse.bacc as bacc
import concourse.mybir as mybir
from concourse import tile
from concourse.bass_utils import run_bass_kernel_spmd

F32 = mybir.dt.float32
F16 = mybir.dt.float16
BF16 = mybir.dt.bfloat16
B, F, NH, NB = 8, 400, 100, 65
SR, BLOCK = 16000, 128
T = F * BLOCK
LOG10 = math.log(10.0)
MAGIC = 12582912.0  # 1.5 * 2**23
NCHUNK = 50         # harmonic chunks of 1024 samples
CHW = 1024

_cache = {}


def _host_constants():
    b = np.arange(NB)[:, None]
    m = np.arange(128)[None, :]
    w = np.where((b == 0) | (b == 64), 1.0, 2.0)
    Cmat = w / 128.0 * np.cos(2 * np.pi * b * m / 128.0)
    win2 = 0.5 + 0.5 * np.cos(2 * np.pi * np.arange(128) / 128.0)
    Air = (2.0 * Cmat * win2[None, :]).astype(np.float32)               # (65,128)
    ccol = (1e-7 * (Cmat * win2[None, :]).sum(0)).astype(np.float32)[:, None]  # (128,1)
    j = np.arange(128)[:, None]
    bb = np.arange(128)[None, :]
    Dcos = np.cos(2 * np.pi * j * bb / 256.0).astype(np.float32)        # (128j,128b)
    Dsin = (-np.sin(2 * np.pi * j * bb / 256.0)).astype(np.float32)
    d128 = np.cos(np.pi * np.arange(128)).astype(np.float32)[:, None]   # (128,1)
    bb2 = np.arange(128)[:, None]
    i = np.arange(128)[None, :]
    cb = np.where(bb2 == 0, 1.0, 2.0)
    ICre = (cb / 256.0 * np.cos(2 * np.pi * bb2 * i / 256.0)).astype(np.float32)
    ICim = (-2.0 / 256.0 * np.sin(2 * np.pi * bb2 * i / 256.0)).astype(np.float32)
    nyq = ((1.0 / 256.0) * np.cos(np.pi * np.arange(128))).astype(np.float32)[None, :]
    # output samples are produced block-reversed (partition p = sample 127-p)
    ICre = np.ascontiguousarray(ICre[:, ::-1])
    ICim = np.ascontiguousarray(ICim[:, ::-1])
    nyq = np.ascontiguousarray(nyq[:, ::-1])
    kvneg = np.zeros((1, 128), np.float32)
    kvneg[0, :NH] = -np.arange(1, NH + 1)
    eye = np.eye(128, dtype=np.float32)
    return dict(c_air=Air, c_ccol=ccol, c_dcos=Dcos, c_dsin=Dsin, c_d128=d128,
                c_icre=ICre, c_icim=ICim, c_nyq=nyq, c_kvneg=kvneg, c_eye=eye)


def _build():
    nc = bacc.Bacc(None, target_bir_lowering=False, debug=False)

    amp_d = nc.dram_tensor("amp_param", [F, NH + 1], F32, kind="ExternalInput")
    npr_d = nc.dram_tensor("noise_param", [F, NB], F32, kind="ExternalInput")
    pit_d = nc.dram_tensor("pitch", [F, 1], F32, kind="ExternalInput")
    noi_d = nc.dram_tensor("noise", [F, BLOCK], F32, kind="ExternalInput")
    rvn_d = nc.dram_tensor("reverb_noise", [SR, 1], F32, kind="ExternalInput")
    dec_d = nc.dram_tensor("decay", [1, 1], F32, kind="ExternalInput")
    wet_d = nc.dram_tensor("wet", [1, 1], F32, kind="ExternalInput")
    cst = {}
    for name, shape in [("c_air", [NB, 128]), ("c_ccol", [128, 1]),
                        ("c_dcos", [128, 128]), ("c_dsin", [128, 128]),
                        ("c_d128", [128, 1]), ("c_icre", [128, 128]),
                        ("c_icim", [128, 128]), ("c_nyq", [1, 128]),
                        ("c_kvneg", [1, 128]), ("c_eye", [128, 128])]:
        cst[name] = nc.dram_tensor(name, shape, F32, kind="ExternalInput")
    out_d = nc.dram_tensor("out", [128, F], F32, kind="ExternalOutput")

    wscr16 = nc.dram_tensor("wscr16", [2, T], BF16)
    hpbuf16 = nc.dram_tensor("hpbuf16", [1, 16384], BF16)

    AF = mybir.ActivationFunctionType
    OP = mybir.AluOpType

    with tile.TileContext(nc) as tc:
        with tc.tile_pool(name="const", bufs=1) as cpool, \
             tc.tile_pool(name="big", bufs=1) as big, \
             tc.tile_pool(name="work", bufs=1) as work, \
             tc.tile_pool(name="chunk", bufs=3) as chk:

            # ---------- constants ----------
            t_mcol = cpool.tile([128, 1], F32)
            nc.vector.memset(t_mcol[:], MAGIC)
            t_eps = cpool.tile([128, 1], F32)
            nc.vector.memset(t_eps[:], 1e-7)
            t_b5 = cpool.tile([128, 1], F32)
            nc.vector.memset(t_b5[:], 5.0)
            t_kv2f = cpool.tile([2, 128], F32)
            nc.sync.dma_start(t_kv2f[:], bass.AP(cst["c_kvneg"], 0, [[0, 2], [1, 128]]))
            t_kv16 = cpool.tile([2, 128], BF16)
            nc.vector.tensor_copy(t_kv16[:], t_kv2f[:])
            t_air = cpool.tile([NB, 128], F32)
            nc.sync.dma_start(t_air[:], cst["c_air"][:])
            t_ccol = cpool.tile([128, 1], F32)
            nc.sync.dma_start(t_ccol[:], cst["c_ccol"][:])
            t_dcos = cpool.tile([128, 128], F32)
            nc.sync.dma_start(t_dcos[:], cst["c_dcos"][:])
            t_dsin = cpool.tile([128, 128], F32)
            nc.sync.dma_start(t_dsin[:], cst["c_dsin"][:])
            t_d128 = cpool.tile([128, 1], F32)
            nc.sync.dma_start(t_d128[:], cst["c_d128"][:])
            t_icre32 = cpool.tile([128, 128], F32)
            nc.sync.dma_start(t_icre32[:], cst["c_icre"][:])
            t_icre16 = cpool.tile([128, 128], BF16)
            nc.vector.tensor_copy(t_icre16[:], t_icre32[:])
            t_icim32 = cpool.tile([128, 128], F32)
            nc.sync.dma_start(t_icim32[:], cst["c_icim"][:])
            t_icim16 = cpool.tile([128, 128], BF16)
            nc.vector.tensor_copy(t_icim16[:], t_icim32[:])
            t_nyq32 = cpool.tile([1, 128], F32)
            nc.sync.dma_start(t_nyq32[:], cst["c_nyq"][:])
            t_nyq16 = cpool.tile([1, 128], BF16)
            nc.vector.tensor_copy(t_nyq16[:], t_nyq32[:])
            t_eye = cpool.tile([128, 128], F32)
            nc.sync.dma_start(t_eye[:], cst["c_eye"][:])
            t_eye16 = cpool.tile([128, 128], BF16)
            nc.vector.tensor_copy(t_eye16[:], t_eye[:])
            t_kroi = cpool.tile([128, NH], mybir.dt.int32)
            nc.gpsimd.iota(t_kroi[:], pattern=[[1, NH]], base=1, channel_multiplier=0)
            t_krow = cpool.tile([128, NH], F32)
            nc.vector.tensor_copy(t_krow[:], t_kroi[:])
            t_jp1i = cpool.tile([128, 1], mybir.dt.int32)
            nc.gpsimd.iota(t_jp1i[:], pattern=[[1, 1]], base=1, channel_multiplier=1)
            t_jp1 = cpool.tile([128, 1], F32)
            nc.vector.tensor_copy(t_jp1[:], t_jp1i[:])
            t_jrev = cpool.tile([128, 1], F32)
            nc.vector.tensor_scalar(out=t_jrev[:], in0=t_jp1[:], scalar1=-1.0,
                                    scalar2=129.0, op0=OP.mult, op1=OP.add)
            t_ioti = cpool.tile([128, 125], mybir.dt.int32)
            nc.gpsimd.iota(t_ioti[:], pattern=[[128, 125]], base=0, channel_multiplier=1)
            t_iotf = cpool.tile([128, 125], F32)
            nc.vector.tensor_copy(t_iotf[:], t_ioti[:])

            # early loads for impulse build
            t_dec = work.tile([1, 1], F32)
            nc.sync.dma_start(t_dec[:], dec_d[:])
            t_wet = work.tile([1, 1], F32)
            nc.sync.dma_start(t_wet[:], wet_d[:])
            t_rn = work.tile([128, 125], F32)
            nc.sync.dma_start(t_rn[:], bass.AP(rvn_d, 0, [[1, 128], [128, 125]]))
            t_noT = big.tile([128, F], F32)
            nc.sync.dma_start(t_noT[:], bass.AP(noi_d, 0, [[1, 128], [128, F]]))
            t_pcol = work.tile([100, 4], F32)
            nc.sync.dma_start(t_pcol[:], bass.AP(pit_d, 0, [[1, 100], [100, 4]]))

            with tc.high_priority():
                # ---------- phase chain (critical path to stage B) ----------
                t_pr = work.tile([1, F], F32)
                nc.sync.dma_start(t_pr[:], bass.AP(pit_d, 0, [[1, 1], [1, F]]))
                t_p8 = work.tile([1, F], F32)
                nc.scalar.activation(t_p8[:], t_pr[:], AF.Identity,
                                     bias=t_mcol[0:1, :], scale=8.0)
                t_ph = work.tile([1, F], F32)
                nc.vector.tensor_scalar(out=t_ph[:], in0=t_p8[:], scalar1=MAGIC,
                                        scalar2=0.125, op0=OP.subtract, op1=OP.mult)
                t_pl = work.tile([1, F], F32)
                nc.vector.tensor_sub(t_pl[:], t_pr[:], t_ph[:])
                t_zr = work.tile([1, F], F32)
                nc.vector.memset(t_zr[:], 0.0)
                t_sh = work.tile([1, F + 1], F32)
                nc.vector.memset(t_sh[:, 0:1], 0.0)
                nc.vector.tensor_tensor_scan(t_sh[:, 1:F + 1], t_ph[:], t_zr[:], 0.0,
                                             OP.add, OP.add)
                t_sl = work.tile([1, F + 1], F32)
                nc.vector.memset(t_sl[:, 0:1], 0.0)
                nc.vector.tensor_tensor_scan(t_sl[:, 1:F + 1], t_pl[:], t_zr[:], 0.0,
                                             OP.add, OP.add)
                t_ts = work.tile([1, F], F32)
                nc.vector.tensor_add(t_ts[:], t_sh[:, 0:F], t_sl[:, 0:F])
                t_t2r = work.tile([1, F], F32)
                nc.scalar.activation(t_t2r[:], t_ts[:], AF.Identity,
                                     bias=t_mcol[0:1, :], scale=1.0 / 125.0)
                t_n125 = work.tile([1, F], F32)
                nc.vector.tensor_scalar(out=t_n125[:], in0=t_t2r[:], scalar1=MAGIC,
                                        scalar2=None, op0=OP.subtract)
                t_u = work.tile([1, F], F32)
                nc.vector.scalar_tensor_tensor(out=t_u[:], in0=t_n125[:], scalar=-125.0,
                                               in1=t_sh[:, 0:F], op0=OP.mult, op1=OP.add)
                t_r125 = work.tile([1, F], F32)
                nc.vector.tensor_add(t_r125[:], t_u[:], t_sl[:, 0:F])
                t_om = work.tile([1, F], F32)
                nc.vector.tensor_scalar(out=t_om[:], in0=t_r125[:], scalar1=1.0 / 125.0,
                                        scalar2=None, op0=OP.mult)
                t_cr = work.tile([1, F], F32)
                nc.vector.tensor_scalar(out=t_cr[:], in0=t_pr[:], scalar1=1.0 / 16000.0,
                                        scalar2=None, op0=OP.mult)
                t_omb = work.tile([128, F], F32)
                nc.gpsimd.partition_broadcast(t_omb[:], t_om[:])
                t_cb = work.tile([128, F], F32)
                nc.gpsimd.partition_broadcast(t_cb[:], t_cr[:])
                t_wraw = work.tile([128, F], F32)
                nc.vector.scalar_tensor_tensor(out=t_wraw[:], in0=t_cb[:],
                                               scalar=t_jrev[:, :], in1=t_omb[:],
                                               op0=OP.mult, op1=OP.add)
                t_tw = work.tile([128, F], F32)
                nc.scalar.activation(t_tw[:], t_wraw[:], AF.Identity,
                                     bias=t_mcol[:, :], scale=1.0)
                t_wneg = work.tile([128, F], F32)
                nc.vector.scalar_tensor_tensor(out=t_wneg[:], in0=t_tw[:], scalar=MAGIC,
                                               in1=t_wraw[:], op0=OP.subtract,
                                               op1=OP.subtract)
                t_whi16 = work.tile([128, F], BF16)
                nc.vector.tensor_copy(t_whi16[:], t_wneg[:])
                t_wlo16 = work.tile([128, F], BF16)
                nc.vector.tensor_sub(t_wlo16[:], t_wneg[:], t_whi16[:])
                with tc.tile_pool(name="pswt", bufs=2, space="PSUM") as pswt:
                    t_wThi = work.tile([100, 512], BF16)
                    t_wTlo = work.tile([100, 512], BF16)
                    for g in range(4):
                        p_th = pswt.tile([100, 128], BF16, tag="wt")
                        nc.tensor.transpose(p_th[:], t_whi16[:, g * 100:(g + 1) * 100],
                                            t_eye16[:])
                        nc.scalar.copy(t_wThi[:, g * 128:(g + 1) * 128], p_th[:])
                        p_tl = pswt.tile([100, 128], BF16, tag="wt")
                        nc.tensor.transpose(p_tl[:], t_wlo16[:, g * 100:(g + 1) * 100],
                                            t_eye16[:])
                        nc.scalar.copy(t_wTlo[:, g * 128:(g + 1) * 128], p_tl[:])
                    # group 0 first (unblocks chunk 0), then groups 1-3 merged;
                    # hi on sync queue, lo on gpsimd queue, in parallel
                    nc.sync.dma_start(
                        bass.AP(wscr16, 0, [[128, 100], [1, 128]]),
                        t_wThi[:, 0:128])
                    nc.gpsimd.dma_start(
                        bass.AP(wscr16, T, [[128, 100], [1, 128]]),
                        t_wTlo[:, 0:128])
                    nc.sync.dma_start(
                        bass.AP(wscr16, 12800, [[128, 100], [12800, 3], [1, 128]]),
                        t_wThi[:, 128:512])
                    nc.gpsimd.dma_start(
                        bass.AP(wscr16, T + 12800, [[128, 100], [12800, 3], [1, 128]]),
                        t_wTlo[:, 128:512])

            # ---------- impulse + Toeplitz gather (gpsimd queue) ----------
            t_e1 = work.tile([1, 1], F32)
            nc.scalar.activation(t_e1[:], t_dec[:], AF.Exp, scale=-1.0)
            t_sp = work.tile([1, 1], F32)
            nc.scalar.activation(t_sp[:], t_e1[:], AF.Ln, bias=1.0, scale=1.0)
            t_s32 = work.tile([1, 1], F32)
            nc.vector.tensor_scalar(out=t_s32[:], in0=t_sp[:], scalar1=-1.0 / 32.0,
                                    scalar2=None, op0=OP.mult)
            t_ew = work.tile([1, 1], F32)
            nc.scalar.activation(t_ew[:], t_wet[:], AF.Exp, scale=-1.0)
            t_ew1 = work.tile([1, 1], F32)
            nc.vector.tensor_scalar(out=t_ew1[:], in0=t_ew[:], scalar1=1.0,
                                    scalar2=None, op0=OP.add)
            t_sw = work.tile([1, 1], F32)
            nc.vector.reciprocal(t_sw[:], t_ew1[:])
            t_s32b = work.tile([128, 1], F32)
            nc.gpsimd.partition_broadcast(t_s32b[:], t_s32[:])
            t_swb = work.tile([128, 1], F32)
            nc.gpsimd.partition_broadcast(t_swb[:], t_sw[:])
            t_env = work.tile([128, 125], F32)
            nc.scalar.activation(t_env[:], t_iotf[:], AF.Exp, scale=t_s32b[:, :])
            t_h = work.tile([128, 125], F32)
            nc.vector.scalar_tensor_tensor(out=t_h[:], in0=t_env[:], scalar=t_swb[:, :],
                                           in1=t_rn[:], op0=OP.mult, op1=OP.mult)
            nc.vector.memset(t_h[0:1, 0:1], 1.0)
            t_h16 = work.tile([128, 125], BF16)
            nc.vector.tensor_copy(t_h16[:], t_h[:])
            t_z16 = work.tile([1, 160], BF16)
            nc.vector.memset(t_z16[:], 0.0)
            nc.gpsimd.dma_start(bass.AP(hpbuf16, 0, [[1, 1], [1, 127]]),
                                t_z16[0:1, 0:127])
            nc.gpsimd.dma_start(bass.AP(hpbuf16, 127, [[1, 128], [128, 125]]), t_h16[:])
            nc.gpsimd.dma_start(bass.AP(hpbuf16, 16127, [[1, 1], [1, 129]]),
                                t_z16[0:1, 0:129])
            t_hs16 = big.tile([128, 16128], BF16)
            for q in range(4):
                nc.gpsimd.dma_start(
                    t_hs16[q * 32:(q + 1) * 32, :],
                    bass.AP(hpbuf16, q * 32, [[1, 32], [128, 126], [1, 128]]))

            with tc.tile_pool(name="pstr", bufs=2, space="PSUM") as pstr:
                # ---------- amp params ----------
                t_at = big.tile([128, F], F32)
                nc.vector.memset(t_at[:], 0.0)
                amp_sf = []
                for g in range(4):
                    f0 = g * 100
                    t_ap = work.tile([100, NH + 1], F32, tag=f"ampchunk{g}", name=f"ampchunk{g}")
                    nc.sync.dma_start(t_ap[:], amp_d[f0:f0 + 100, :])
                    t_sf = work.tile([100, NH + 1], F32, tag=f"ampsf{g}", name=f"ampsf{g}")
                    nc.scalar.activation(t_sf[:], t_ap[:], AF.Exp, scale=-1.0)
                    amp_sf.append(t_sf)
                ns_sf = []
                for g in range(4):
                    f0 = g * 100
                    t_np = work.tile([100, NB], F32, tag=f"npchunk{g}", name=f"npchunk{g}")
                    nc.sync.dma_start(t_np[:], npr_d[f0:f0 + 100, :])
                    t_ns = work.tile([100, NB], F32, tag=f"nsf{g}", name=f"nsf{g}")
                    nc.scalar.activation(t_ns[:], t_np[:], AF.Exp, bias=t_b5[0:100, :], scale=-1.0)
                    ns_sf.append(t_ns)
                for t_sf in amp_sf + ns_sf:
                    nc.scalar.activation(t_sf[:], t_sf[:], AF.Ln, bias=1.0, scale=1.0)
                for t_sf in amp_sf + ns_sf:
                    nc.scalar.activation(t_sf[:], t_sf[:], AF.Exp, scale=-LOG10)
                for t_sf in amp_sf:
                    nc.scalar.activation(t_sf[:], t_sf[:], AF.Identity,
                                         bias=t_eps[0:100, :], scale=2.0)
                for g in range(4):
                    f0 = g * 100
                    t_sf = amp_sf[g]
                    t_kp = work.tile([100, NH], F32, tag="kp")
                    nc.vector.tensor_scalar(out=t_kp[:], in0=t_krow[0:100, :],
                                            scalar1=t_pcol[:, g:g + 1], scalar2=None, op0=OP.mult)
                    t_aa = work.tile([100, NH], F32, tag="aa")
                    nc.vector.tensor_scalar(out=t_aa[:], in0=t_kp[:], scalar1=8000.0,
                                            scalar2=1e-4, op0=OP.is_lt, op1=OP.add)
                    t_am = work.tile([100, NH], F32, tag="am")
                    nc.vector.tensor_mul(t_am[:], t_sf[:, 1:NH + 1], t_aa[:])
                    t_ssum = work.tile([100, 1], F32, tag="ssum")
                    nc.vector.tensor_reduce(out=t_ssum[:], in_=t_am[:],
                                            axis=mybir.AxisListType.X, op=OP.add)
                    t_rec = work.tile([100, 1], F32, tag="rec")
                    nc.vector.reciprocal(t_rec[:], t_ssum[:])
                    t_scn = work.tile([100, 1], F32, tag="scn")
                    nc.vector.tensor_mul(t_scn[:], t_rec[:], t_sf[:, 0:1])
                    nc.vector.tensor_scalar(out=t_scn[:], in0=t_scn[:], scalar1=-1.0,
                                            scalar2=None, op0=OP.mult)
                    nc.vector.tensor_scalar(out=t_am[:], in0=t_am[:], scalar1=t_scn[:, :],
                                            scalar2=None, op0=OP.mult)
                    p_tr = pstr.tile([100, 100], F32, tag="tr")
                    nc.tensor.transpose(p_tr[:], t_am[:], t_eye[0:100, 0:100])
                    nc.scalar.copy(t_at[0:100, f0:f0 + 100], p_tr[:])
                t_at16 = big.tile([128, F], F16)
                nc.vector.tensor_copy(t_at16[:], t_at[:])

                # ---------- noise branch ----------
                t_nt = big.tile([65, F], F32)
                for g in range(4):
                    f0 = g * 100
                    p_tr2 = pstr.tile([65, 100], F32, tag="tr")
                    nc.tensor.transpose(p_tr2[:], ns_sf[g][:], t_eye[0:100, 0:100])
                    nc.scalar.copy(t_nt[0:65, f0:f0 + 100], p_tr2[:])
                with tc.tile_pool(name="psn", bufs=1, space="PSUM") as psn:
                    p_irp = psn.tile([128, F], F32, tag="tmp", bufs=2)
                    nc.tensor.matmul(p_irp[:], t_air[:], t_nt[0:65, :], start=True, stop=True)
                    t_irp = big.tile([128, F], F32)
                    nc.vector.tensor_scalar(out=t_irp[:], in0=p_irp[:],
                                            scalar1=t_ccol[:, :], scalar2=None, op0=OP.add)
                    p_hre = psn.tile([128, F], F32, tag="tmp", bufs=2)
                    nc.tensor.matmul(p_hre[:], t_dcos[:], t_irp[:], start=True, stop=True)
                    t_hre = big.tile([128, F], F32)
                    nc.scalar.copy(t_hre[:], p_hre[:])
                    p_him = psn.tile([128, F], F32, tag="tmp", bufs=2)
                    nc.tensor.matmul(p_him[:], t_dsin[:], t_irp[:], start=True, stop=True)
                    t_him = big.tile([128, F], F32)
                    nc.scalar.copy(t_him[:], p_him[:])
                    p_h128 = psn.tile([1, F], F32, tag="tmp", bufs=2)
                    nc.tensor.matmul(p_h128[:], t_d128[:], t_irp[:], start=True, stop=True)
                    t_h128 = big.tile([1, F], F32)
                    nc.scalar.copy(t_h128[:], p_h128[:])
                    p_nre = psn.tile([128, F], F32, tag="nre")
                    nc.tensor.matmul(p_nre[:], t_dcos[:], t_noT[:], start=True, stop=True)
                    p_nim = psn.tile([128, F], F32, tag="nim")
                    nc.tensor.matmul(p_nim[:], t_dsin[:], t_noT[:], start=True, stop=True)
                    p_n128 = psn.tile([1, F], F32, tag="tmp", bufs=2)
                    nc.tensor.matmul(p_n128[:], t_d128[:], t_noT[:], start=True, stop=True)
                    t_a = work.tile([128, F], F32, tag="pa")
                    nc.vector.tensor_mul(t_a[:], t_hre[:], p_nre[:])
                    t_b = work.tile([128, F], F32, tag="pb")
                    nc.vector.tensor_mul(t_b[:], t_him[:], p_nim[:])
                    t_pre = big.tile([128, F], BF16)
                    nc.vector.tensor_sub(t_pre[:], t_a[:], t_b[:])
                    t_c2 = work.tile([128, F], F32, tag="pc2")
                    nc.vector.tensor_mul(t_c2[:], t_him[:], p_nre[:])
                    t_d2 = work.tile([128, F], F32, tag="pd2")
                    nc.vector.tensor_mul(t_d2[:], t_hre[:], p_nim[:])
                    t_pim = big.tile([128, F], BF16)
                    nc.vector.tensor_add(t_pim[:], t_c2[:], t_d2[:])
                    t_p128 = big.tile([1, F], BF16)
                    nc.vector.tensor_mul(t_p128[:], t_h128[:], p_n128[:])

            # ---------- harmonic stage B: depth-4 software pipeline ----------
            with tc.tile_pool(name="pskw", bufs=3, space="PSUM") as pskw, \
                 tc.tile_pool(name="psO", bufs=1, space="PSUM") as psO, \
                 tc.tile_pool(name="psnz2", bufs=1, space="PSUM") as psnz:
                p_nz = psnz.tile([128, F], F32)
                nc.tensor.matmul(p_nz[:], t_icre16[:], t_pre[:], start=True, stop=False)
                nc.tensor.matmul(p_nz[:], t_icim16[:], t_pim[:], start=False, stop=False)
                nc.tensor.matmul(p_nz[:], t_nyq16[:], t_p128[:], start=False, stop=True)

                p_O = psO.tile([128, F], F32)
                kws, t1s, svs, sns = {}, {}, {}, {}
                for c in range(NCHUNK + 4):
                    if c < NCHUNK:
                        t_wc = chk.tile([2, CHW], BF16, tag="wc")
                        nc.sync.dma_start(t_wc[:],
                                          bass.AP(wscr16, c * CHW, [[T, 2], [1, CHW]]))
                        p_kw = pskw.tile([128, CHW], F32, tag="kw")
                        for half in range(2):
                            nc.tensor.matmul(p_kw[:, half * 512:(half + 1) * 512],
                                             t_kv16[:], t_wc[:, half * 512:(half + 1) * 512],
                                             start=True, stop=True)
                        kws[c] = p_kw
                    if c - 1 >= 0 and c - 1 < NCHUNK:
                        cj = c - 1
                        pk = kws[cj]
                        t_t1 = chk.tile([128, CHW], F32, tag="t1")
                        if cj % 5 in (1, 3):
                            nc.vector.tensor_scalar(out=t_t1[:], in0=pk[:], scalar1=MAGIC,
                                                    scalar2=None, op0=OP.add)
                        else:
                            nc.scalar.activation(t_t1[:], pk[:], AF.Identity,
                                                 bias=t_mcol[:, :], scale=1.0)
                        t1s[cj] = t_t1
                    if c - 2 >= 0 and c - 2 < NCHUNK:
                        t_sv = chk.tile([128, CHW], F32, tag="sv")
                        nc.vector.scalar_tensor_tensor(out=t_sv[:], in0=t1s.pop(c - 2)[:],
                                                       scalar=MAGIC, in1=kws.pop(c - 2)[:],
                                                       op0=OP.subtract, op1=OP.subtract)
                        svs[c - 2] = t_sv
                    if c - 3 >= 0 and c - 3 < NCHUNK:
                        t_sn = chk.tile([128, CHW], F16, tag="sn")
                        nc.scalar.activation(t_sn[:], svs.pop(c - 3)[:], AF.Sin,
                                             scale=2.0 * math.pi)
                        sns[c - 3] = t_sn
                    if c - 4 >= 0:
                        cj = c - 4
                        t_sn = sns.pop(cj)
                        for fl in range(8):
                            f = 8 * cj + fl
                            nc.tensor.matmul(p_O[:, f:f + 1],
                                             t_sn[:, fl * 128:(fl + 1) * 128],
                                             t_at16[:, f:f + 1], start=True, stop=True)

                # ---------- combine (block-reversed samples) ----------
                t_spad16 = big.tile([128, 526], BF16)
                nc.vector.memset(t_spad16[:, 0:126], 0.0)
                nc.scalar.copy(t_spad16[:, 126:526], p_O[:])
                nc.vector.tensor_add(t_spad16[:, 126:526], t_spad16[:, 126:526], p_nz[:])

            # ---------- reverb: 126 accumulating bf16 Toeplitz matmuls ----------
            with tc.tile_pool(name="psrev", bufs=1, space="PSUM") as psr:
                p_rev = psr.tile([128, F], F32)
                for d in range(126):
                    nc.tensor.matmul(p_rev[:], t_hs16[:, d * 128:d * 128 + 128],
                                     t_spad16[:, 126 - d:526 - d],
                                     start=(d == 0), stop=(d == 125))
                t_out = big.tile([128, F], F32)
                nc.scalar.copy(t_out[:], p_rev[:])
                nc.sync.dma_start(out_d[:], t_out[:])

    nc.compile()
    return nc


def kernel(**inputs):
    if "nc" not in _cache:
        _cache["nc"] = _build()
        _cache["consts"] = _host_constants()
    nc = _cache["nc"]
    consts = _cache["consts"]

    amp = np.ascontiguousarray(np.asarray(inputs["amp_param"], np.float32))
    npr = np.ascontiguousarray(np.asarray(inputs["noise_param"], np.float32))
    pit = np.ascontiguousarray(np.asarray(inputs["pitch"], np.float32))
    noi = np.ascontiguousarray(np.asarray(inputs["noise"], np.float32))
    rvn = np.ascontiguousarray(np.asarray(inputs["reverb_noise"], np.float32))
    dec = np.asarray(inputs["decay"], np.float32).reshape(1, 1)
    wet = np.asarray(inputs["wet"], np.float32).reshape(1, 1)

    in_maps = []
    for b in range(B):
        m = dict(amp_param=amp[b], noise_param=npr[b], pitch=pit[b],
                 noise=noi[b], reverb_noise=rvn, decay=dec, wet=wet)
        m.update(consts)
        in_maps.append(m)

    res = run_bass_kernel_spmd(nc, in_maps, list(range(B)))
    out = np.stack([res.results[b]["out"].T.reshape(T, 1) for b in range(B)])
    return out.astype(np.float32)


if __name__ == "__main__":
    rng = np.random.default_rng(0)
    ins = dict(
        amp_param=rng.standard_normal((B, F, NH + 1)).astype(np.float32),
        noise_param=rng.standard_normal((B, F, NB)).astype(np.float32),
        pitch=(rng.random((B, F, 1), np.float32) * 440 + 60),
        noise=(rng.random((B, F, BLOCK), np.float32) * 2 - 1),
        reverb_noise=(rng.random((SR, 1), np.float32) * 2 - 1),
        decay=np.ones(1, np.float32) * 5,
        wet=np.zeros(1, np.float32),
        sampling_rate=SR, block_size=BLOCK,
    )
    o = kernel(**ins)
    print("kernel out", o.shape, o.dtype, np.abs(o).max())


def _install_ntff_hook():
    import sys as _sys
    import types as _types
    try:
        import antenv.axon_hooks  # noqa: F401
        return
    except ImportError:
        pass
    from trn_agent_boot.trn_boot import _ntff_profile_via_ctypes
    hook = _ntff_profile_via_ctypes('/opt/axon/libaxon_pjrt.so')
    mod = _types.ModuleType('antenv.axon_hooks')
    _h = {'v': hook}
    mod.get_axon_ntff_profile_hook = lambda: _h['v']
    mod.set_axon_ntff_profile_hook = lambda h: _h.update(v=h)
    _sys.modules['antenv.axon_hooks'] = mod
    import antenv
    antenv.axon_hooks = mod


def run_timed(**inputs):
    """Re-run with NTFF tracing enabled; returns max per-core exec ns or None."""
    _install_ntff_hook()
    if "nc" not in _cache:
        _cache["nc"] = _build()
        _cache["consts"] = _host_constants()
    nc = _cache["nc"]
    consts = _cache["consts"]
    amp = np.ascontiguousarray(np.asarray(inputs["amp_param"], np.float32))
    npr = np.ascontiguousarray(np.asarray(inputs["noise_param"], np.float32))
    pit = np.ascontiguousarray(np.asarray(inputs["pitch"], np.float32))
    noi = np.ascontiguousarray(np.asarray(inputs["noise"], np.float32))
    rvn = np.ascontiguousarray(np.asarray(inputs["reverb_noise"], np.float32))
    dec = np.asarray(inputs["decay"], np.float32).reshape(1, 1)
    wet = np.asarray(inputs["wet"], np.float32).reshape(1, 1)
    in_maps = []
    for b in range(B):
        m = dict(amp_param=amp[b], noise_param=npr[b], pitch=pit[b],
                 noise=noi[b], reverb_noise=rvn, decay=dec, wet=wet)
        m.update(consts)
        in_maps.append(m)
    res = run_bass_kernel_spmd(nc, in_maps, list(range(B)), trace=True)
    if res.instructions_and_trace is not None:
        _cache["insts"] = res.instructions_and_trace[0]
    return res.exec_time_ns


# revision 8
# speedup vs baseline: 3.2723x; 1.1553x over previous
# DDSP synthesizer kernel for Trainium2 (8 NeuronCores, batch-parallel).
#
# Per core (one batch element):
#   harmonic branch: exact-phase oscillator bank. Phases are wrapped to
#     [-0.5,0.5] in (j,f) layout, split hi/lo bf16, PE-transposed to
#     time-major, then k*w computed as a K=2 bf16 outer-product matmul
#     (exact products, fp32 PSUM accumulate). Magic-number wrap
#     (ScalarE/DVE split), ScalarE Sin, per-frame weighted reduction on PE.
#   noise branch: irfft+window folded into one DFT matmul, frame-wise
#     128-tap causal conv via rfft-256 as PE matmuls (inverse side bf16).
#   reverb: 16000-tap causal FIR as 126 accumulating bf16 Toeplitz matmuls;
#     the Toeplitz operand is fetched with 4 wide positive-stride DMAs
#     (samples are generated block-reversed so the shift structure needs
#     no negative strides).
import math
import numpy as np

import concourse.bass as bass
import concourse.bacc as bacc
import concourse.mybir as mybir
from concourse import tile
from concourse.bass_utils import run_bass_kernel_spmd

F32 = mybir.dt.float32
F16 = mybir.dt.float16
BF16 = mybir.dt.bfloat16
B, F, NH, NB = 8, 400, 100, 65
SR, BLOCK = 16000, 128
T = F * BLOCK
LOG10 = math.log(10.0)
MAGIC = 12582912.0  # 1.5 * 2**23
NCHUNK = 50         # harmonic chunks of 1024 samples
CHW = 1024

_cache = {}


def _host_constants():
    b = np.arange(NB)[:, None]
    m = np.arange(128)[None, :]
    w = np.where((b == 0) | (b == 64), 1.0, 2.0)
    Cmat = w / 128.0 * np.cos(2 * np.pi * b * m / 128.0)
    win2 = 0.5 + 0.5 * np.cos(2 * np.pi * np.arange(128) / 128.0)
    Air = (2.0 * Cmat * win2[None, :]).astype(np.float32)               # (65,128)
    ccol = (1e-7 * (Cmat * win2[None, :]).sum(0)).astype(np.float32)[:, None]  # (128,1)
    j = np.arange(128)[:, None]
    bb = np.arange(128)[None, :]
    Dcos = np.cos(2 * np.pi * j * bb / 256.0).astype(np.float32)        # (128j,128b)
    Dsin = (-np.sin(2 * np.pi * j * bb / 256.0)).astype(np.float32)
    d128 = np.cos(np.pi * np.arange(128)).astype(np.float32)[:, None]   # (128,1)
    bb2 = np.arange(128)[:, None]
    i = np.arange(128)[None, :]
    cb = np.where(bb2 == 0, 1.0, 2.0)
    ICre = (cb / 256.0 * np.cos(2 * np.pi * bb2 * i / 256.0)).astype(np.float32)
    ICim = (-2.0 / 256.0 * np.sin(2 * np.pi * bb2 * i / 256.0)).astype(np.float32)
    nyq = ((1.0 / 256.0) * np.cos(np.pi * np.arange(128))).astype(np.float32)[None, :]
    # output samples are produced block-reversed (partition p = sample 127-p)
    ICre = np.ascontiguousarray(ICre[:, ::-1])
    ICim = np.ascontiguousarray(ICim[:, ::-1])
    nyq = np.ascontiguousarray(nyq[:, ::-1])
    kvneg = np.zeros((1, 128), np.float32)
    kvneg[0, :NH] = -np.arange(1, NH + 1)
    eye = np.eye(128, dtype=np.float32)
    return dict(c_air=Air, c_ccol=ccol, c_dcos=Dcos, c_dsin=Dsin, c_d128=d128,
                c_icre=ICre, c_icim=ICim, c_nyq=nyq, c_kvneg=kvneg, c_eye=eye)


def _build():
    nc = bacc.Bacc(None, target_bir_lowering=False, debug=False)

    amp_d = nc.dram_tensor("amp_param", [F, NH + 1], F32, kind="ExternalInput")
    npr_d = nc.dram_tensor("noise_param", [F, NB], F32, kind="ExternalInput")
    pit_d = nc.dram_tensor("pitch", [F, 1], F32, kind="ExternalInput")
    noi_d = nc.dram_tensor("noise", [F, BLOCK], F32, kind="ExternalInput")
    rvn_d = nc.dram_tensor("reverb_noise", [SR, 1], F32, kind="ExternalInput")
    dec_d = nc.dram_tensor("decay", [1, 1], F32, kind="ExternalInput")
    wet_d = nc.dram_tensor("wet", [1, 1], F32, kind="ExternalInput")
    cst = {}
    for name, shape in [("c_air", [NB, 128]), ("c_ccol", [128, 1]),
                        ("c_dcos", [128, 128]), ("c_dsin", [128, 128]),
                        ("c_d128", [128, 1]), ("c_icre", [128, 128]),
                        ("c_icim", [128, 128]), ("c_nyq", [1, 128]),
                        ("c_kvneg", [1, 128]), ("c_eye", [128, 128])]:
        cst[name] = nc.dram_tensor(name, shape, F32, kind="ExternalInput")
    out_d = nc.dram_tensor("out", [128, F], F32, kind="ExternalOutput")

    wscr16 = nc.dram_tensor("wscr16", [2, T], BF16)
    hpbuf16 = nc.dram_tensor("hpbuf16", [1, 16384], BF16)

    AF = mybir.ActivationFunctionType
    OP = mybir.AluOpType

    with tile.TileContext(nc) as tc:
        with tc.tile_pool(name="const", bufs=1) as cpool, \
             tc.tile_pool(name="big", bufs=1) as big, \
             tc.tile_pool(name="work", bufs=1) as work, \
             tc.tile_pool(name="chunk", bufs=3) as chk:

            # ---------- constants ----------
            t_mcol = cpool.tile([128, 1], F32)
            nc.vector.memset(t_mcol[:], MAGIC)
            t_eps = cpool.tile([128, 1], F32)
            nc.vector.memset(t_eps[:], 1e-7)
            t_b5 = cpool.tile([128, 1], F32)
            nc.vector.memset(t_b5[:], 5.0)
            t_kv2f = cpool.tile([2, 128], F32)
            nc.sync.dma_start(t_kv2f[:], bass.AP(cst["c_kvneg"], 0, [[0, 2], [1, 128]]))
            t_kv16 = cpool.tile([2, 128], BF16)
            nc.vector.tensor_copy(t_kv16[:], t_kv2f[:])
            t_air = cpool.tile([NB, 128], F32)
            nc.sync.dma_start(t_air[:], cst["c_air"][:])
            t_ccol = cpool.tile([128, 1], F32)
            nc.sync.dma_start(t_ccol[:], cst["c_ccol"][:])
            t_dcos = cpool.tile([128, 128], F32)
            nc.sync.dma_start(t_dcos[:], cst["c_dcos"][:])
            t_dsin = cpool.tile([128, 128], F32)
            nc.sync.dma_start(t_dsin[:], cst["c_dsin"][:])
            t_d128 = cpool.tile([128, 1], F32)
            nc.sync.dma_start(t_d128[:], cst["c_d128"][:])
            t_icre32 = cpool.tile([128, 128], F32)
            nc.sync.dma_start(t_icre32[:], cst["c_icre"][:])
            t_icre16 = cpool.tile([128, 128], BF16)
            nc.vector.tensor_copy(t_icre16[:], t_icre32[:])
            t_icim32 = cpool.tile([128, 128], F32)
            nc.sync.dma_start(t_icim32[:], cst["c_icim"][:])
            t_icim16 = cpool.tile([128, 128], BF16)
            nc.vector.tensor_copy(t_icim16[:], t_icim32[:])
            t_nyq32 = cpool.tile([1, 128], F32)
            nc.sync.dma_start(t_nyq32[:], cst["c_nyq"][:])
            t_nyq16 = cpool.tile([1, 128], BF16)
            nc.vector.tensor_copy(t_nyq16[:], t_nyq32[:])
            t_eye = cpool.tile([128, 128], F32)
            nc.sync.dma_start(t_eye[:], cst["c_eye"][:])
            t_eye16 = cpool.tile([128, 128], BF16)
            nc.vector.tensor_copy(t_eye16[:], t_eye[:])
            t_kroi = cpool.tile([128, NH], mybir.dt.int32)
            nc.gpsimd.iota(t_kroi[:], pattern=[[1, NH]], base=1, channel_multiplier=0)
            t_krow = cpool.tile([128, NH], F32)
            nc.vector.tensor_copy(t_krow[:], t_kroi[:])
            t_jp1i = cpool.tile([128, 1], mybir.dt.int32)
            nc.gpsimd.iota(t_jp1i[:], pattern=[[1, 1]], base=1, channel_multiplier=1)
            t_jp1 = cpool.tile([128, 1], F32)
            nc.vector.tensor_copy(t_jp1[:], t_jp1i[:])
            t_jrev = cpool.tile([128, 1], F32)
            nc.vector.tensor_scalar(out=t_jrev[:], in0=t_jp1[:], scalar1=-1.0,
                                    scalar2=129.0, op0=OP.mult, op1=OP.add)
            t_ioti = cpool.tile([128, 125], mybir.dt.int32)
            nc.gpsimd.iota(t_ioti[:], pattern=[[128, 125]], base=0, channel_multiplier=1)
            t_iotf = cpool.tile([128, 125], F32)
            nc.vector.tensor_copy(t_iotf[:], t_ioti[:])

            # early loads for impulse build
            t_dec = work.tile([1, 1], F32)
            nc.sync.dma_start(t_dec[:], dec_d[:])
            t_wet = work.tile([1, 1], F32)
            nc.sync.dma_start(t_wet[:], wet_d[:])
            t_rn = work.tile([128, 125], F32)
            nc.sync.dma_start(t_rn[:], bass.AP(rvn_d, 0, [[1, 128], [128, 125]]))

            # ---------- impulse + Toeplitz gather (gpsimd queue) ----------
            t_e1 = work.tile([1, 1], F32)
            nc.scalar.activation(t_e1[:], t_dec[:], AF.Exp, scale=-1.0)
            t_sp = work.tile([1, 1], F32)
            nc.scalar.activation(t_sp[:], t_e1[:], AF.Ln, bias=1.0, scale=1.0)
            t_s32 = work.tile([1, 1], F32)
            nc.vector.tensor_scalar(out=t_s32[:], in0=t_sp[:], scalar1=-1.0 / 32.0,
                                    scalar2=None, op0=OP.mult)
            t_ew = work.tile([1, 1], F32)
            nc.scalar.activation(t_ew[:], t_wet[:], AF.Exp, scale=-1.0)
            t_ew1 = work.tile([1, 1], F32)
            nc.vector.tensor_scalar(out=t_ew1[:], in0=t_ew[:], scalar1=1.0,
                                    scalar2=None, op0=OP.add)
            t_sw = work.tile([1, 1], F32)
            nc.vector.reciprocal(t_sw[:], t_ew1[:])
            t_s32b = work.tile([128, 1], F32)
            nc.gpsimd.partition_broadcast(t_s32b[:], t_s32[:])
            t_swb = work.tile([128, 1], F32)
            nc.gpsimd.partition_broadcast(t_swb[:], t_sw[:])
            t_env = work.tile([128, 125], F32)
            nc.scalar.activation(t_env[:], t_iotf[:], AF.Exp, scale=t_s32b[:, :])
            t_h = work.tile([128, 125], F32)
            nc.vector.scalar_tensor_tensor(out=t_h[:], in0=t_env[:], scalar=t_swb[:, :],
                                           in1=t_rn[:], op0=OP.mult, op1=OP.mult)
            nc.vector.memset(t_h[0:1, 0:1], 1.0)
            t_h16 = work.tile([128, 125], BF16)
            nc.vector.tensor_copy(t_h16[:], t_h[:])
            t_z16 = work.tile([1, 160], BF16)
            nc.vector.memset(t_z16[:], 0.0)
            nc.gpsimd.dma_start(bass.AP(hpbuf16, 0, [[1, 1], [1, 127]]),
                                t_z16[0:1, 0:127])
            nc.gpsimd.dma_start(bass.AP(hpbuf16, 127, [[1, 128], [128, 125]]), t_h16[:])
            nc.gpsimd.dma_start(bass.AP(hpbuf16, 16127, [[1, 1], [1, 129]]),
                                t_z16[0:1, 0:129])
            t_hs16 = big.tile([128, 16128], BF16)
            for q in range(4):
                nc.gpsimd.dma_start(
                    t_hs16[q * 32:(q + 1) * 32, :],
                    bass.AP(hpbuf16, q * 32, [[1, 32], [128, 126], [1, 128]]))

            with tc.high_priority():
                # ---------- phase chain (critical path to stage B) ----------
                t_pr = work.tile([1, F], F32)
                nc.sync.dma_start(t_pr[:], bass.AP(pit_d, 0, [[1, 1], [1, F]]))
                t_p8 = work.tile([1, F], F32)
                nc.scalar.activation(t_p8[:], t_pr[:], AF.Identity,
                                     bias=t_mcol[0:1, :], scale=8.0)
                t_ph = work.tile([1, F], F32)
                nc.vector.tensor_scalar(out=t_ph[:], in0=t_p8[:], scalar1=MAGIC,
                                        scalar2=0.125, op0=OP.subtract, op1=OP.mult)
                t_pl = work.tile([1, F], F32)
                nc.vector.tensor_sub(t_pl[:], t_pr[:], t_ph[:])
                t_zr = work.tile([1, F], F32)
                nc.vector.memset(t_zr[:], 0.0)
                t_sh = work.tile([1, F + 1], F32)
                nc.vector.memset(t_sh[:, 0:1], 0.0)
                nc.vector.tensor_tensor_scan(t_sh[:, 1:F + 1], t_ph[:], t_zr[:], 0.0,
                                             OP.add, OP.add)
                t_sl = work.tile([1, F + 1], F32)
                nc.vector.memset(t_sl[:, 0:1], 0.0)
                nc.vector.tensor_tensor_scan(t_sl[:, 1:F + 1], t_pl[:], t_zr[:], 0.0,
                                             OP.add, OP.add)
                t_ts = work.tile([1, F], F32)
                nc.vector.tensor_add(t_ts[:], t_sh[:, 0:F], t_sl[:, 0:F])
                t_t2r = work.tile([1, F], F32)
                nc.scalar.activation(t_t2r[:], t_ts[:], AF.Identity,
                                     bias=t_mcol[0:1, :], scale=1.0 / 125.0)
                t_n125 = work.tile([1, F], F32)
                nc.vector.tensor_scalar(out=t_n125[:], in0=t_t2r[:], scalar1=MAGIC,
                                        scalar2=None, op0=OP.subtract)
                t_u = work.tile([1, F], F32)
                nc.vector.scalar_tensor_tensor(out=t_u[:], in0=t_n125[:], scalar=-125.0,
                                               in1=t_sh[:, 0:F], op0=OP.mult, op1=OP.add)
                t_r125 = work.tile([1, F], F32)
                nc.vector.tensor_add(t_r125[:], t_u[:], t_sl[:, 0:F])
                t_om = work.tile([1, F], F32)
                nc.vector.tensor_scalar(out=t_om[:], in0=t_r125[:], scalar1=1.0 / 125.0,
                                        scalar2=None, op0=OP.mult)
                t_cr = work.tile([1, F], F32)
                nc.vector.tensor_scalar(out=t_cr[:], in0=t_pr[:], scalar1=1.0 / 16000.0,
                                        scalar2=None, op0=OP.mult)
                t_omb = work.tile([128, F], F32)
                nc.gpsimd.partition_broadcast(t_omb[:], t_om[:])
                t_cb = work.tile([128, F], F32)
                nc.gpsimd.partition_broadcast(t_cb[:], t_cr[:])
                t_wraw = work.tile([128, F], F32)
                nc.vector.scalar_tensor_tensor(out=t_wraw[:], in0=t_cb[:],
                                               scalar=t_jrev[:, :], in1=t_omb[:],
                                               op0=OP.mult, op1=OP.add)
                t_tw = work.tile([128, F], F32)
                nc.scalar.activation(t_tw[:], t_wraw[:], AF.Identity,
                                     bias=t_mcol[:, :], scale=1.0)
                t_wneg = work.tile([128, F], F32)
                nc.vector.scalar_tensor_tensor(out=t_wneg[:], in0=t_tw[:], scalar=MAGIC,
                                               in1=t_wraw[:], op0=OP.subtract,
                                               op1=OP.subtract)
                t_whi16 = work.tile([128, F], BF16)
                nc.vector.tensor_copy(t_whi16[:], t_wneg[:])
                t_wlo16 = work.tile([128, F], BF16)
                nc.vector.tensor_sub(t_wlo16[:], t_wneg[:], t_whi16[:])
                with tc.tile_pool(name="pswt", bufs=2, space="PSUM") as pswt:
                    t_wThi = work.tile([100, 512], BF16)
                    t_wTlo = work.tile([100, 512], BF16)
                    for g in range(4):
                        p_th = pswt.tile([100, 128], BF16, tag="wt")
                        nc.tensor.transpose(p_th[:], t_whi16[:, g * 100:(g + 1) * 100],
                                            t_eye16[:])
                        nc.scalar.copy(t_wThi[:, g * 128:(g + 1) * 128], p_th[:])
                        p_tl = pswt.tile([100, 128], BF16, tag="wt")
                        nc.tensor.transpose(p_tl[:], t_wlo16[:, g * 100:(g + 1) * 100],
                                            t_eye16[:])
                        nc.scalar.copy(t_wTlo[:, g * 128:(g + 1) * 128], p_tl[:])
                    # group 0 first (unblocks chunk 0), then groups 1-3 merged;
                    # hi on sync queue, lo on gpsimd queue, in parallel
                    nc.sync.dma_start(
                        bass.AP(wscr16, 0, [[128, 100], [1, 128]]),
                        t_wThi[:, 0:128])
                    nc.gpsimd.dma_start(
                        bass.AP(wscr16, T, [[128, 100], [1, 128]]),
                        t_wTlo[:, 0:128])
                    nc.sync.dma_start(
                        bass.AP(wscr16, 12800, [[128, 100], [12800, 3], [1, 128]]),
                        t_wThi[:, 128:512])
                    nc.gpsimd.dma_start(
                        bass.AP(wscr16, T + 12800, [[128, 100], [12800, 3], [1, 128]]),
                        t_wTlo[:, 128:512])

            with tc.tile_pool(name="pstr", bufs=2, space="PSUM") as pstr:
                # ---------- amp params ----------
                t_at = big.tile([128, F], F32)
                nc.vector.memset(t_at[:], 0.0)
                amp_sf = []
                for g in range(4):
                    f0 = g * 100
                    t_ap = work.tile([100, NH + 1], F32, tag=f"ampchunk{g}", name=f"ampchunk{g}")
                    nc.sync.dma_start(t_ap[:], amp_d[f0:f0 + 100, :])
                    t_sf = work.tile([100, NH + 1], F32, tag=f"ampsf{g}", name=f"ampsf{g}")
                    nc.scalar.activation(t_sf[:], t_ap[:], AF.Exp, scale=-1.0)
                    amp_sf.append(t_sf)
                ns_sf = []
                for g in range(4):
                    f0 = g * 100
                    t_np = work.tile([100, NB], F32, tag=f"npchunk{g}", name=f"npchunk{g}")
                    nc.sync.dma_start(t_np[:], npr_d[f0:f0 + 100, :])
                    t_ns = work.tile([100, NB], F32, tag=f"nsf{g}", name=f"nsf{g}")
                    nc.scalar.activation(t_ns[:], t_np[:], AF.Exp, bias=t_b5[0:100, :], scale=-1.0)
                    ns_sf.append(t_ns)
                for t_sf in amp_sf + ns_sf:
                    nc.scalar.activation(t_sf[:], t_sf[:], AF.Ln, bias=1.0, scale=1.0)
                for t_sf in amp_sf + ns_sf:
                    nc.scalar.activation(t_sf[:], t_sf[:], AF.Exp, scale=-LOG10)
                for t_sf in amp_sf:
                    nc.scalar.activation(t_sf[:], t_sf[:], AF.Identity,
                                         bias=t_eps[0:100, :], scale=2.0)
                t_pc4 = work.tile([4, 100], F32)
                nc.sync.dma_start(t_pc4[:], bass.AP(pit_d, 0, [[100, 4], [1, 100]]))
                p_pc = pstr.tile([100, 4], F32, tag="tr")
                nc.tensor.transpose(p_pc[:], t_pc4[:], t_eye[0:4, 0:4])
                t_pcol = work.tile([100, 4], F32)
                nc.scalar.copy(t_pcol[:], p_pc[:])
                for g in range(4):
                    f0 = g * 100
                    t_sf = amp_sf[g]
                    t_kp = work.tile([100, NH], F32, tag="kp")
                    nc.vector.tensor_scalar(out=t_kp[:], in0=t_krow[0:100, :],
                                            scalar1=t_pcol[:, g:g + 1], scalar2=None, op0=OP.mult)
                    t_aa = work.tile([100, NH], F32, tag="aa")
                    nc.vector.tensor_scalar(out=t_aa[:], in0=t_kp[:], scalar1=8000.0,
                                            scalar2=1e-4, op0=OP.is_lt, op1=OP.add)
                    t_am = work.tile([100, NH], F32, tag="am")
                    nc.vector.tensor_mul(t_am[:], t_sf[:, 1:NH + 1], t_aa[:])
                    t_ssum = work.tile([100, 1], F32, tag="ssum")
                    nc.vector.tensor_reduce(out=t_ssum[:], in_=t_am[:],
                                            axis=mybir.AxisListType.X, op=OP.add)
                    t_rec = work.tile([100, 1], F32, tag="rec")
                    nc.vector.reciprocal(t_rec[:], t_ssum[:])
                    t_scn = work.tile([100, 1], F32, tag="scn")
                    nc.vector.tensor_mul(t_scn[:], t_rec[:], t_sf[:, 0:1])
                    nc.vector.tensor_scalar(out=t_scn[:], in0=t_scn[:], scalar1=-1.0,
                                            scalar2=None, op0=OP.mult)
                    nc.vector.tensor_scalar(out=t_am[:], in0=t_am[:], scalar1=t_scn[:, :],
                                            scalar2=None, op0=OP.mult)
                    p_tr = pstr.tile([100, 100], F32, tag="tr")
                    nc.tensor.transpose(p_tr[:], t_am[:], t_eye[0:100, 0:100])
                    nc.scalar.copy(t_at[0:100, f0:f0 + 100], p_tr[:])
                t_at16 = big.tile([128, F], F16)
                nc.vector.tensor_copy(t_at16[:], t_at[:])

                # ---------- noise branch ----------
                t_nt = big.tile([65, F], F32)
                for g in range(4):
                    f0 = g * 100
                    p_tr2 = pstr.tile([65, 100], F32, tag="tr")
                    nc.tensor.transpose(p_tr2[:], ns_sf[g][:], t_eye[0:100, 0:100])
                    nc.scalar.copy(t_nt[0:65, f0:f0 + 100], p_tr2[:])
                t_noT = big.tile([128, F], F32)
                for g in range(4):
                    t_nog = work.tile([100, 128], F32, tag=f"nog{g}", name=f"nog{g}")
                    nc.sync.dma_start(t_nog[:], noi_d[g * 100:(g + 1) * 100, :])
                    p_not = pstr.tile([128, 100], F32, tag="tr")
                    nc.tensor.transpose(p_not[:], t_nog[:], t_eye[0:100, 0:100])
                    nc.scalar.copy(t_noT[:, g * 100:(g + 1) * 100], p_not[:])

                with tc.tile_pool(name="psn", bufs=1, space="PSUM") as psn:
                    p_irp = psn.tile([128, F], F32, tag="tmp", bufs=2)
                    nc.tensor.matmul(p_irp[:], t_air[:], t_nt[0:65, :], start=True, stop=True)
                    t_irp = big.tile([128, F], F32)
                    nc.vector.tensor_scalar(out=t_irp[:], in0=p_irp[:],
                                            scalar1=t_ccol[:, :], scalar2=None, op0=OP.add)
                    p_hre = psn.tile([128, F], F32, tag="tmp", bufs=2)
                    nc.tensor.matmul(p_hre[:], t_dcos[:], t_irp[:], start=True, stop=True)
                    t_hre = big.tile([128, F], F32)
                    nc.scalar.copy(t_hre[:], p_hre[:])
                    p_him = psn.tile([128, F], F32, tag="tmp", bufs=2)
                    nc.tensor.matmul(p_him[:], t_dsin[:], t_irp[:], start=True, stop=True)
                    t_him = big.tile([128, F], F32)
                    nc.scalar.copy(t_him[:], p_him[:])
                    p_h128 = psn.tile([1, F], F32, tag="tmp", bufs=2)
                    nc.tensor.matmul(p_h128[:], t_d128[:], t_irp[:], start=True, stop=True)
                    t_h128 = big.tile([1, F], F32)
                    nc.scalar.copy(t_h128[:], p_h128[:])
                    p_nre = psn.tile([128, F], F32, tag="nre")
                    nc.tensor.matmul(p_nre[:], t_dcos[:], t_noT[:], start=True, stop=True)
                    p_nim = psn.tile([128, F], F32, tag="nim")
                    nc.tensor.matmul(p_nim[:], t_dsin[:], t_noT[:], start=True, stop=True)
                    p_n128 = psn.tile([1, F], F32, tag="tmp", bufs=2)
                    nc.tensor.matmul(p_n128[:], t_d128[:], t_noT[:], start=True, stop=True)
                    t_a = work.tile([128, F], F32, tag="pa")
                    nc.vector.tensor_mul(t_a[:], t_hre[:], p_nre[:])
                    t_b = work.tile([128, F], F32, tag="pb")
                    nc.vector.tensor_mul(t_b[:], t_him[:], p_nim[:])
                    t_pre = big.tile([128, F], BF16)
                    nc.vector.tensor_sub(t_pre[:], t_a[:], t_b[:])
                    t_c2 = work.tile([128, F], F32, tag="pc2")
                    nc.vector.tensor_mul(t_c2[:], t_him[:], p_nre[:])
                    t_d2 = work.tile([128, F], F32, tag="pd2")
                    nc.vector.tensor_mul(t_d2[:], t_hre[:], p_nim[:])
                    t_pim = big.tile([128, F], BF16)
                    nc.vector.tensor_add(t_pim[:], t_c2[:], t_d2[:])
                    t_p128 = big.tile([1, F], BF16)
                    nc.vector.tensor_mul(t_p128[:], t_h128[:], p_n128[:])

            # ---------- harmonic stage B: depth-4 software pipeline ----------
            with tc.tile_pool(name="pskw", bufs=3, space="PSUM") as pskw, \
                 tc.tile_pool(name="psO", bufs=1, space="PSUM") as psO, \
                 tc.tile_pool(name="psnz2", bufs=1, space="PSUM") as psnz:
                p_nz = psnz.tile([128, F], F32)
                nc.tensor.matmul(p_nz[:], t_icre16[:], t_pre[:], start=True, stop=False)
                nc.tensor.matmul(p_nz[:], t_icim16[:], t_pim[:], start=False, stop=False)
                nc.tensor.matmul(p_nz[:], t_nyq16[:], t_p128[:], start=False, stop=True)

                p_O = psO.tile([128, F], F32)
                kws, t1s, svs, sns = {}, {}, {}, {}
                for c in range(NCHUNK + 4):
                    if c < NCHUNK:
                        t_wc = chk.tile([2, CHW], BF16, tag="wc")
                        nc.sync.dma_start(t_wc[:],
                                          bass.AP(wscr16, c * CHW, [[T, 2], [1, CHW]]))
                        p_kw = pskw.tile([128, CHW], F32, tag="kw")
                        for half in range(2):
                            nc.tensor.matmul(p_kw[:, half * 512:(half + 1) * 512],
                                             t_kv16[:], t_wc[:, half * 512:(half + 1) * 512],
                                             start=True, stop=True)
                        kws[c] = p_kw
                    if c - 1 >= 0 and c - 1 < NCHUNK:
                        cj = c - 1
                        pk = kws[cj]
                        t_t1 = chk.tile([128, CHW], F32, tag="t1")
                        if cj % 5 in (1, 3):
                            nc.vector.tensor_scalar(out=t_t1[:], in0=pk[:], scalar1=MAGIC,
                                                    scalar2=None, op0=OP.add)
                        else:
                            nc.scalar.activation(t_t1[:], pk[:], AF.Identity,
                                                 bias=t_mcol[:, :], scale=1.0)
                        t1s[cj] = t_t1
                    if c - 2 >= 0 and c - 2 < NCHUNK:
                        t_sv = chk.tile([128, CHW], F32, tag="sv")
                        nc.vector.scalar_tensor_tensor(out=t_sv[:], in0=t1s.pop(c - 2)[:],
                                                       scalar=MAGIC, in1=kws.pop(c - 2)[:],
                                                       op0=OP.subtract, op1=OP.subtract)
                        svs[c - 2] = t_sv
                    if c - 3 >= 0 and c - 3 < NCHUNK:
                        t_sn = chk.tile([128, CHW], F16, tag="sn")
                        nc.scalar.activation(t_sn[:], svs.pop(c - 3)[:], AF.Sin,
                                             scale=2.0 * math.pi)
                        sns[c - 3] = t_sn
                    if c - 4 >= 0:
                        cj = c - 4
                        t_sn = sns.pop(cj)
                        for fl in range(8):
                            f = 8 * cj + fl
                            nc.tensor.matmul(p_O[:, f:f + 1],
                                             t_sn[:, fl * 128:(fl + 1) * 128],
                                             t_at16[:, f:f + 1], start=True, stop=True)

                # ---------- combine (block-reversed samples) ----------
                t_spad16 = big.tile([128, 526], BF16)
                nc.vector.memset(t_spad16[:, 0:126], 0.0)
                nc.scalar.copy(t_spad16[:, 126:526], p_O[:])
                nc.vector.tensor_add(t_spad16[:, 126:526], t_spad16[:, 126:526], p_nz[:])

            # ---------- reverb: 126 accumulating bf16 Toeplitz matmuls ----------
            with tc.tile_pool(name="psrev", bufs=1, space="PSUM") as psr:
                p_rev = psr.tile([128, F], F32)
                for d in range(126):
                    nc.tensor.matmul(p_rev[:], t_hs16[:, d * 128:d * 128 + 128],
                                     t_spad16[:, 126 - d:526 - d],
                                     start=(d == 0), stop=(d == 125))
                t_out = big.tile([128, F], F32)
                nc.scalar.copy(t_out[:], p_rev[:])
                nc.sync.dma_start(out_d[:], t_out[:])

    nc.compile()
    return nc


def kernel(**inputs):
    if "nc" not in _cache:
        _cache["nc"] = _build()
        _cache["consts"] = _host_constants()
    nc = _cache["nc"]
    consts = _cache["consts"]

    amp = np.ascontiguousarray(np.asarray(inputs["amp_param"], np.float32))
    npr = np.ascontiguousarray(np.asarray(inputs["noise_param"], np.float32))
    pit = np.ascontiguousarray(np.asarray(inputs["pitch"], np.float32))
    noi = np.ascontiguousarray(np.asarray(inputs["noise"], np.float32))
    rvn = np.ascontiguousarray(np.asarray(inputs["reverb_noise"], np.float32))
    dec = np.asarray(inputs["decay"], np.float32).reshape(1, 1)
    wet = np.asarray(inputs["wet"], np.float32).reshape(1, 1)

    in_maps = []
    for b in range(B):
        m = dict(amp_param=amp[b], noise_param=npr[b], pitch=pit[b],
                 noise=noi[b], reverb_noise=rvn, decay=dec, wet=wet)
        m.update(consts)
        in_maps.append(m)

    res = run_bass_kernel_spmd(nc, in_maps, list(range(B)))
    out = np.stack([res.results[b]["out"].T.reshape(T, 1) for b in range(B)])
    return out.astype(np.float32)


if __name__ == "__main__":
    rng = np.random.default_rng(0)
    ins = dict(
        amp_param=rng.standard_normal((B, F, NH + 1)).astype(np.float32),
        noise_param=rng.standard_normal((B, F, NB)).astype(np.float32),
        pitch=(rng.random((B, F, 1), np.float32) * 440 + 60),
        noise=(rng.random((B, F, BLOCK), np.float32) * 2 - 1),
        reverb_noise=(rng.random((SR, 1), np.float32) * 2 - 1),
        decay=np.ones(1, np.float32) * 5,
        wet=np.zeros(1, np.float32),
        sampling_rate=SR, block_size=BLOCK,
    )
    o = kernel(**ins)
    print("kernel out", o.shape, o.dtype, np.abs(o).max())


def _install_ntff_hook():
    import sys as _sys
    import types as _types
    try:
        import antenv.axon_hooks  # noqa: F401
        return
    except ImportError:
        pass
    from trn_agent_boot.trn_boot import _ntff_profile_via_ctypes
    hook = _ntff_profile_via_ctypes('/opt/axon/libaxon_pjrt.so')
    mod = _types.ModuleType('antenv.axon_hooks')
    _h = {'v': hook}
    mod.get_axon_ntff_profile_hook = lambda: _h['v']
    mod.set_axon_ntff_profile_hook = lambda h: _h.update(v=h)
    _sys.modules['antenv.axon_hooks'] = mod
    import antenv
    antenv.axon_hooks = mod


def run_timed(**inputs):
    """Re-run with NTFF tracing enabled; returns max per-core exec ns or None."""
    _install_ntff_hook()
    if "nc" not in _cache:
        _cache["nc"] = _build()
        _cache["consts"] = _host_constants()
    nc = _cache["nc"]
    consts = _cache["consts"]
    amp = np.ascontiguousarray(np.asarray(inputs["amp_param"], np.float32))
    npr = np.ascontiguousarray(np.asarray(inputs["noise_param"], np.float32))
    pit = np.ascontiguousarray(np.asarray(inputs["pitch"], np.float32))
    noi = np.ascontiguousarray(np.asarray(inputs["noise"], np.float32))
    rvn = np.ascontiguousarray(np.asarray(inputs["reverb_noise"], np.float32))
    dec = np.asarray(inputs["decay"], np.float32).reshape(1, 1)
    wet = np.asarray(inputs["wet"], np.float32).reshape(1, 1)
    in_maps = []
    for b in range(B):
        m = dict(amp_param=amp[b], noise_param=npr[b], pitch=pit[b],
                 noise=noi[b], reverb_noise=rvn, decay=dec, wet=wet)
        m.update(consts)
        in_maps.append(m)
    res = run_bass_kernel_spmd(nc, in_maps, list(range(B)), trace=True)
    if res.instructions_and_trace is not None:
        _cache["insts"] = res.instructions_and_trace[0]
    return res.exec_time_ns


# revision 9
# speedup vs baseline: 3.6040x; 1.1014x over previous
# DDSP synthesizer kernel for Trainium2 (8 NeuronCores, batch-parallel).
#
# Per core (one batch element):
#   harmonic branch: exact-phase oscillator bank. Phases are wrapped to
#     [-0.5,0.5] in (j,f) layout, split hi/lo bf16, PE-transposed to
#     time-major, then k*w computed as a K=2 bf16 outer-product matmul
#     (exact products, fp32 PSUM accumulate). Magic-number wrap
#     (ScalarE/DVE split), ScalarE Sin, per-frame weighted reduction on PE.
#   noise branch: irfft+window folded into one DFT matmul, frame-wise
#     128-tap causal conv via rfft-256 as PE matmuls (inverse side bf16).
#   reverb: 16000-tap causal FIR as 126 accumulating bf16 Toeplitz matmuls;
#     the Toeplitz operand is fetched with 4 wide positive-stride DMAs
#     (samples are generated block-reversed so the shift structure needs
#     no negative strides).
import math
import numpy as np

import concourse.bass as bass
import concourse.bacc as bacc
import concourse.mybir as mybir
from concourse import tile
from concourse.bass_utils import run_bass_kernel_spmd

F32 = mybir.dt.float32
F16 = mybir.dt.float16
BF16 = mybir.dt.bfloat16
B, F, NH, NB = 8, 400, 100, 65
SR, BLOCK = 16000, 128
T = F * BLOCK
LOG10 = math.log(10.0)
MAGIC = 12582912.0  # 1.5 * 2**23
NCHUNK = 50         # harmonic chunks of 1024 samples
CHW = 1024

_cache = {}


def _host_constants():
    b = np.arange(NB)[:, None]
    m = np.arange(128)[None, :]
    w = np.where((b == 0) | (b == 64), 1.0, 2.0)
    Cmat = w / 128.0 * np.cos(2 * np.pi * b * m / 128.0)
    win2 = 0.5 + 0.5 * np.cos(2 * np.pi * np.arange(128) / 128.0)
    Air = (2.0 * Cmat * win2[None, :]).astype(np.float32)               # (65,128)
    ccol = (1e-7 * (Cmat * win2[None, :]).sum(0)).astype(np.float32)[:, None]  # (128,1)
    j = np.arange(128)[:, None]
    bb = np.arange(128)[None, :]
    Dcos = np.cos(2 * np.pi * j * bb / 256.0).astype(np.float32)        # (128j,128b)
    Dsin = (-np.sin(2 * np.pi * j * bb / 256.0)).astype(np.float32)
    d128 = np.cos(np.pi * np.arange(128)).astype(np.float32)[:, None]   # (128,1)
    bb2 = np.arange(128)[:, None]
    i = np.arange(128)[None, :]
    cb = np.where(bb2 == 0, 1.0, 2.0)
    ICre = (cb / 256.0 * np.cos(2 * np.pi * bb2 * i / 256.0)).astype(np.float32)
    ICim = (-2.0 / 256.0 * np.sin(2 * np.pi * bb2 * i / 256.0)).astype(np.float32)
    nyq = ((1.0 / 256.0) * np.cos(np.pi * np.arange(128))).astype(np.float32)[None, :]
    # output samples are produced block-reversed (partition p = sample 127-p)
    ICre = np.ascontiguousarray(ICre[:, ::-1])
    ICim = np.ascontiguousarray(ICim[:, ::-1])
    nyq = np.ascontiguousarray(nyq[:, ::-1])
    kvneg = np.zeros((1, 128), np.float32)
    kvneg[0, :NH] = -np.arange(1, NH + 1)
    eye = np.eye(128, dtype=np.float32)
    return dict(c_air=Air, c_ccol=ccol, c_dcos=Dcos, c_dsin=Dsin, c_d128=d128,
                c_icre=ICre, c_icim=ICim, c_nyq=nyq, c_kvneg=kvneg, c_eye=eye)


def _build():
    nc = bacc.Bacc(None, target_bir_lowering=False, debug=False)

    amp_d = nc.dram_tensor("amp_param", [F, NH + 1], F32, kind="ExternalInput")
    npr_d = nc.dram_tensor("noise_param", [F, NB], F32, kind="ExternalInput")
    pit_d = nc.dram_tensor("pitch", [F, 1], F32, kind="ExternalInput")
    noi_d = nc.dram_tensor("noise", [F, BLOCK], F32, kind="ExternalInput")
    rvn_d = nc.dram_tensor("reverb_noise", [SR, 1], F32, kind="ExternalInput")
    dec_d = nc.dram_tensor("decay", [1, 1], F32, kind="ExternalInput")
    wet_d = nc.dram_tensor("wet", [1, 1], F32, kind="ExternalInput")
    cst = {}
    for name, shape in [("c_air", [NB, 128]), ("c_ccol", [128, 1]),
                        ("c_dcos", [128, 128]), ("c_dsin", [128, 128]),
                        ("c_d128", [128, 1]), ("c_icre", [128, 128]),
                        ("c_icim", [128, 128]), ("c_nyq", [1, 128]),
                        ("c_kvneg", [1, 128]), ("c_eye", [128, 128])]:
        cst[name] = nc.dram_tensor(name, shape, F32, kind="ExternalInput")
    out_d = nc.dram_tensor("out", [128, F], F32, kind="ExternalOutput")

    wscr16 = nc.dram_tensor("wscr16", [2, T], BF16)
    hpbuf16 = nc.dram_tensor("hpbuf16", [1, 16384], BF16)

    AF = mybir.ActivationFunctionType
    OP = mybir.AluOpType

    with tile.TileContext(nc) as tc:
        with tc.tile_pool(name="const", bufs=1) as cpool, \
             tc.tile_pool(name="big", bufs=1) as big, \
             tc.tile_pool(name="work", bufs=1) as work, \
             tc.tile_pool(name="chunk", bufs=3) as chk:

            # ---------- constants ----------
            t_mcol = cpool.tile([128, 1], F32)
            nc.vector.memset(t_mcol[:], MAGIC)
            t_eps = cpool.tile([128, 1], F32)
            nc.vector.memset(t_eps[:], 1e-7)
            t_b5 = cpool.tile([128, 1], F32)
            nc.vector.memset(t_b5[:], 5.0)
            t_kv2f = cpool.tile([2, 128], F32)
            nc.sync.dma_start(t_kv2f[:], bass.AP(cst["c_kvneg"], 0, [[0, 2], [1, 128]]))
            t_kv16 = cpool.tile([2, 128], BF16)
            nc.vector.tensor_copy(t_kv16[:], t_kv2f[:])
            t_air = cpool.tile([NB, 128], F32)
            nc.sync.dma_start(t_air[:], cst["c_air"][:])
            t_ccol = cpool.tile([128, 1], F32)
            nc.sync.dma_start(t_ccol[:], cst["c_ccol"][:])
            t_dcos = cpool.tile([128, 128], F32)
            nc.sync.dma_start(t_dcos[:], cst["c_dcos"][:])
            t_dsin = cpool.tile([128, 128], F32)
            nc.sync.dma_start(t_dsin[:], cst["c_dsin"][:])
            t_d128 = cpool.tile([128, 1], F32)
            nc.sync.dma_start(t_d128[:], cst["c_d128"][:])
            t_icre32 = cpool.tile([128, 128], F32)
            nc.sync.dma_start(t_icre32[:], cst["c_icre"][:])
            t_icre16 = cpool.tile([128, 128], BF16)
            nc.vector.tensor_copy(t_icre16[:], t_icre32[:])
            t_icim32 = cpool.tile([128, 128], F32)
            nc.sync.dma_start(t_icim32[:], cst["c_icim"][:])
            t_icim16 = cpool.tile([128, 128], BF16)
            nc.vector.tensor_copy(t_icim16[:], t_icim32[:])
            t_nyq32 = cpool.tile([1, 128], F32)
            nc.sync.dma_start(t_nyq32[:], cst["c_nyq"][:])
            t_nyq16 = cpool.tile([1, 128], BF16)
            nc.vector.tensor_copy(t_nyq16[:], t_nyq32[:])
            t_eye = cpool.tile([128, 128], F32)
            nc.sync.dma_start(t_eye[:], cst["c_eye"][:])
            t_eye16 = cpool.tile([128, 128], BF16)
            nc.vector.tensor_copy(t_eye16[:], t_eye[:])
            t_kroi = cpool.tile([128, NH], mybir.dt.int32)
            nc.gpsimd.iota(t_kroi[:], pattern=[[1, NH]], base=1, channel_multiplier=0)
            t_krow = cpool.tile([128, NH], F32)
            nc.vector.tensor_copy(t_krow[:], t_kroi[:])
            t_jp1i = cpool.tile([128, 1], mybir.dt.int32)
            nc.gpsimd.iota(t_jp1i[:], pattern=[[1, 1]], base=1, channel_multiplier=1)
            t_jp1 = cpool.tile([128, 1], F32)
            nc.vector.tensor_copy(t_jp1[:], t_jp1i[:])
            t_jrev = cpool.tile([128, 1], F32)
            nc.vector.tensor_scalar(out=t_jrev[:], in0=t_jp1[:], scalar1=-1.0,
                                    scalar2=129.0, op0=OP.mult, op1=OP.add)
            t_ioti = cpool.tile([128, 125], mybir.dt.int32)
            nc.gpsimd.iota(t_ioti[:], pattern=[[128, 125]], base=0, channel_multiplier=1)
            t_iotf = cpool.tile([128, 125], F32)
            nc.vector.tensor_copy(t_iotf[:], t_ioti[:])

            # early loads for impulse build
            t_dec = work.tile([1, 1], F32)
            nc.sync.dma_start(t_dec[:], dec_d[:])
            t_wet = work.tile([1, 1], F32)
            nc.sync.dma_start(t_wet[:], wet_d[:])
            t_rn = work.tile([128, 125], F32)
            nc.sync.dma_start(t_rn[:], bass.AP(rvn_d, 0, [[1, 128], [128, 125]]))

            with tc.high_priority():
                # ---------- phase chain (critical path to stage B) ----------
                t_pr = work.tile([1, F], F32)
                nc.sync.dma_start(t_pr[:], bass.AP(pit_d, 0, [[1, 1], [1, F]]))
                t_p8 = work.tile([1, F], F32)
                nc.scalar.activation(t_p8[:], t_pr[:], AF.Identity,
                                     bias=t_mcol[0:1, :], scale=8.0)
                t_ph = work.tile([1, F], F32)
                nc.vector.tensor_scalar(out=t_ph[:], in0=t_p8[:], scalar1=MAGIC,
                                        scalar2=0.125, op0=OP.subtract, op1=OP.mult)
                t_pl = work.tile([1, F], F32)
                nc.vector.tensor_sub(t_pl[:], t_pr[:], t_ph[:])
                t_zr = work.tile([1, F], F32)
                nc.vector.memset(t_zr[:], 0.0)
                t_sh = work.tile([1, F + 1], F32)
                nc.vector.memset(t_sh[:, 0:1], 0.0)
                nc.vector.tensor_tensor_scan(t_sh[:, 1:F + 1], t_ph[:], t_zr[:], 0.0,
                                             OP.add, OP.add)
                t_sl = work.tile([1, F + 1], F32)
                nc.vector.memset(t_sl[:, 0:1], 0.0)
                nc.vector.tensor_tensor_scan(t_sl[:, 1:F + 1], t_pl[:], t_zr[:], 0.0,
                                             OP.add, OP.add)
                t_ts = work.tile([1, F], F32)
                nc.vector.tensor_add(t_ts[:], t_sh[:, 0:F], t_sl[:, 0:F])
                t_t2r = work.tile([1, F], F32)
                nc.scalar.activation(t_t2r[:], t_ts[:], AF.Identity,
                                     bias=t_mcol[0:1, :], scale=1.0 / 125.0)
                t_n125 = work.tile([1, F], F32)
                nc.vector.tensor_scalar(out=t_n125[:], in0=t_t2r[:], scalar1=MAGIC,
                                        scalar2=None, op0=OP.subtract)
                t_u = work.tile([1, F], F32)
                nc.vector.scalar_tensor_tensor(out=t_u[:], in0=t_n125[:], scalar=-125.0,
                                               in1=t_sh[:, 0:F], op0=OP.mult, op1=OP.add)
                t_r125 = work.tile([1, F], F32)
                nc.vector.tensor_add(t_r125[:], t_u[:], t_sl[:, 0:F])
                t_om = work.tile([1, F], F32)
                nc.vector.tensor_scalar(out=t_om[:], in0=t_r125[:], scalar1=1.0 / 125.0,
                                        scalar2=None, op0=OP.mult)
                t_cr = work.tile([1, F], F32)
                nc.vector.tensor_scalar(out=t_cr[:], in0=t_pr[:], scalar1=1.0 / 16000.0,
                                        scalar2=None, op0=OP.mult)
                t_omb = work.tile([128, F], F32)
                nc.gpsimd.partition_broadcast(t_omb[:], t_om[:])
                t_cb = work.tile([128, F], F32)
                nc.gpsimd.partition_broadcast(t_cb[:], t_cr[:])
                t_wraw = work.tile([128, F], F32)
                nc.vector.scalar_tensor_tensor(out=t_wraw[:], in0=t_cb[:],
                                               scalar=t_jrev[:, :], in1=t_omb[:],
                                               op0=OP.mult, op1=OP.add)
                t_tw = work.tile([128, F], F32)
                nc.scalar.activation(t_tw[:], t_wraw[:], AF.Identity,
                                     bias=t_mcol[:, :], scale=1.0)
                t_wneg = work.tile([128, F], F32)
                nc.vector.scalar_tensor_tensor(out=t_wneg[:], in0=t_tw[:], scalar=MAGIC,
                                               in1=t_wraw[:], op0=OP.subtract,
                                               op1=OP.subtract)
                t_whi16 = work.tile([128, F], BF16)
                nc.vector.tensor_copy(t_whi16[:], t_wneg[:])
                t_wlo16 = work.tile([128, F], BF16)
                nc.vector.tensor_sub(t_wlo16[:], t_wneg[:], t_whi16[:])
                with tc.tile_pool(name="pswt", bufs=2, space="PSUM") as pswt:
                    t_wThi = work.tile([100, 512], BF16)
                    t_wTlo = work.tile([100, 512], BF16)
                    for g in range(4):
                        p_th = pswt.tile([100, 128], BF16, tag="wt")
                        nc.tensor.transpose(p_th[:], t_whi16[:, g * 100:(g + 1) * 100],
                                            t_eye16[:])
                        nc.scalar.copy(t_wThi[:, g * 128:(g + 1) * 128], p_th[:])
                        p_tl = pswt.tile([100, 128], BF16, tag="wt")
                        nc.tensor.transpose(p_tl[:], t_wlo16[:, g * 100:(g + 1) * 100],
                                            t_eye16[:])
                        nc.scalar.copy(t_wTlo[:, g * 128:(g + 1) * 128], p_tl[:])
                    # group 0 first (unblocks chunk 0), then groups 1-3 merged;
                    # hi on sync queue, lo on gpsimd queue, in parallel
                    nc.sync.dma_start(
                        bass.AP(wscr16, 0, [[128, 100], [1, 128]]),
                        t_wThi[:, 0:128])
                    nc.gpsimd.dma_start(
                        bass.AP(wscr16, T, [[128, 100], [1, 128]]),
                        t_wTlo[:, 0:128])
                    nc.sync.dma_start(
                        bass.AP(wscr16, 12800, [[128, 100], [12800, 3], [1, 128]]),
                        t_wThi[:, 128:512])
                    nc.gpsimd.dma_start(
                        bass.AP(wscr16, T + 12800, [[128, 100], [12800, 3], [1, 128]]),
                        t_wTlo[:, 128:512])

            # ---------- impulse + Toeplitz gather (gpsimd queue) ----------
            t_e1 = work.tile([1, 1], F32)
            nc.scalar.activation(t_e1[:], t_dec[:], AF.Exp, scale=-1.0)
            t_sp = work.tile([1, 1], F32)
            nc.scalar.activation(t_sp[:], t_e1[:], AF.Ln, bias=1.0, scale=1.0)
            t_s32 = work.tile([1, 1], F32)
            nc.vector.tensor_scalar(out=t_s32[:], in0=t_sp[:], scalar1=-1.0 / 32.0,
                                    scalar2=None, op0=OP.mult)
            t_ew = work.tile([1, 1], F32)
            nc.scalar.activation(t_ew[:], t_wet[:], AF.Exp, scale=-1.0)
            t_ew1 = work.tile([1, 1], F32)
            nc.vector.tensor_scalar(out=t_ew1[:], in0=t_ew[:], scalar1=1.0,
                                    scalar2=None, op0=OP.add)
            t_sw = work.tile([1, 1], F32)
            nc.vector.reciprocal(t_sw[:], t_ew1[:])
            t_s32b = work.tile([128, 1], F32)
            nc.gpsimd.partition_broadcast(t_s32b[:], t_s32[:])
            t_swb = work.tile([128, 1], F32)
            nc.gpsimd.partition_broadcast(t_swb[:], t_sw[:])
            t_env = work.tile([128, 125], F32)
            nc.scalar.activation(t_env[:], t_iotf[:], AF.Exp, scale=t_s32b[:, :])
            t_h = work.tile([128, 125], F32)
            nc.vector.scalar_tensor_tensor(out=t_h[:], in0=t_env[:], scalar=t_swb[:, :],
                                           in1=t_rn[:], op0=OP.mult, op1=OP.mult)
            nc.vector.memset(t_h[0:1, 0:1], 1.0)
            with tc.tile_pool(name="psimp", bufs=1, space="PSUM") as psimp:
                p_ht = psimp.tile([125, 128], F32)
                nc.tensor.transpose(p_ht[:], t_h[:], t_eye[:])
                t_hT16 = work.tile([125, 128], BF16)
                nc.scalar.copy(t_hT16[:], p_ht[:])
            t_z16 = work.tile([1, 160], BF16)
            nc.vector.memset(t_z16[:], 0.0)
            nc.gpsimd.dma_start(bass.AP(hpbuf16, 0, [[1, 1], [1, 127]]),
                                t_z16[0:1, 0:127])
            nc.gpsimd.dma_start(bass.AP(hpbuf16, 127, [[128, 125], [1, 128]]),
                                t_hT16[:])
            nc.gpsimd.dma_start(bass.AP(hpbuf16, 16127, [[1, 1], [1, 129]]),
                                t_z16[0:1, 0:129])
            t_hs16 = big.tile([128, 16128], BF16)
            for q in range(4):
                nc.gpsimd.dma_start(
                    t_hs16[q * 32:(q + 1) * 32, :],
                    bass.AP(hpbuf16, q * 32, [[1, 32], [128, 126], [1, 128]]))

            with tc.tile_pool(name="pstr", bufs=2, space="PSUM") as pstr:
                # ---------- amp params ----------
                t_at = big.tile([128, F], F32)
                nc.vector.memset(t_at[:], 0.0)
                amp_sf = []
                for g in range(4):
                    f0 = g * 100
                    t_ap = work.tile([100, NH + 1], F32, tag=f"ampchunk{g}", name=f"ampchunk{g}")
                    nc.sync.dma_start(t_ap[:], amp_d[f0:f0 + 100, :])
                    t_sf = work.tile([100, NH + 1], F32, tag=f"ampsf{g}", name=f"ampsf{g}")
                    nc.scalar.activation(t_sf[:], t_ap[:], AF.Exp, scale=-1.0)
                    amp_sf.append(t_sf)
                ns_sf = []
                for g in range(4):
                    f0 = g * 100
                    t_np = work.tile([100, NB], F32, tag=f"npchunk{g}", name=f"npchunk{g}")
                    nc.sync.dma_start(t_np[:], npr_d[f0:f0 + 100, :])
                    t_ns = work.tile([100, NB], F32, tag=f"nsf{g}", name=f"nsf{g}")
                    nc.scalar.activation(t_ns[:], t_np[:], AF.Exp, bias=t_b5[0:100, :], scale=-1.0)
                    ns_sf.append(t_ns)
                for t_sf in amp_sf + ns_sf:
                    nc.scalar.activation(t_sf[:], t_sf[:], AF.Ln, bias=1.0, scale=1.0)
                for t_sf in amp_sf + ns_sf:
                    nc.scalar.activation(t_sf[:], t_sf[:], AF.Exp, scale=-LOG10)
                for t_sf in amp_sf:
                    nc.scalar.activation(t_sf[:], t_sf[:], AF.Identity,
                                         bias=t_eps[0:100, :], scale=2.0)
                t_pc4 = work.tile([4, 100], F32)
                nc.sync.dma_start(t_pc4[:], bass.AP(pit_d, 0, [[100, 4], [1, 100]]))
                p_pc = pstr.tile([100, 4], F32, tag="tr")
                nc.tensor.transpose(p_pc[:], t_pc4[:], t_eye[0:4, 0:4])
                t_pcol = work.tile([100, 4], F32)
                nc.scalar.copy(t_pcol[:], p_pc[:])
                for g in range(4):
                    f0 = g * 100
                    t_sf = amp_sf[g]
                    t_kp = work.tile([100, NH], F32, tag="kp")
                    nc.vector.tensor_scalar(out=t_kp[:], in0=t_krow[0:100, :],
                                            scalar1=t_pcol[:, g:g + 1], scalar2=None, op0=OP.mult)
                    t_aa = work.tile([100, NH], F32, tag="aa")
                    nc.vector.tensor_scalar(out=t_aa[:], in0=t_kp[:], scalar1=8000.0,
                                            scalar2=1e-4, op0=OP.is_lt, op1=OP.add)
                    t_am = work.tile([100, NH], F32, tag="am")
                    nc.vector.tensor_mul(t_am[:], t_sf[:, 1:NH + 1], t_aa[:])
                    t_ssum = work.tile([100, 1], F32, tag="ssum")
                    nc.vector.tensor_reduce(out=t_ssum[:], in_=t_am[:],
                                            axis=mybir.AxisListType.X, op=OP.add)
                    t_rec = work.tile([100, 1], F32, tag="rec")
                    nc.vector.reciprocal(t_rec[:], t_ssum[:])
                    t_scn = work.tile([100, 1], F32, tag="scn")
                    nc.vector.tensor_mul(t_scn[:], t_rec[:], t_sf[:, 0:1])
                    nc.vector.tensor_scalar(out=t_scn[:], in0=t_scn[:], scalar1=-1.0,
                                            scalar2=None, op0=OP.mult)
                    nc.vector.tensor_scalar(out=t_am[:], in0=t_am[:], scalar1=t_scn[:, :],
                                            scalar2=None, op0=OP.mult)
                    p_tr = pstr.tile([100, 100], F32, tag="tr")
                    nc.tensor.transpose(p_tr[:], t_am[:], t_eye[0:100, 0:100])
                    nc.scalar.copy(t_at[0:100, f0:f0 + 100], p_tr[:])
                t_at16 = big.tile([128, F], F16)
                nc.vector.tensor_copy(t_at16[:], t_at[:])

                # ---------- noise branch ----------
                t_nt = big.tile([65, F], F32)
                for g in range(4):
                    f0 = g * 100
                    p_tr2 = pstr.tile([65, 100], F32, tag="tr")
                    nc.tensor.transpose(p_tr2[:], ns_sf[g][:], t_eye[0:100, 0:100])
                    nc.scalar.copy(t_nt[0:65, f0:f0 + 100], p_tr2[:])
                t_noT = big.tile([128, F], F32)
                for g in range(4):
                    t_nog = work.tile([100, 128], F32, tag=f"nog{g}", name=f"nog{g}")
                    nc.sync.dma_start(t_nog[:], noi_d[g * 100:(g + 1) * 100, :])
                    p_not = pstr.tile([128, 100], F32, tag="tr")
                    nc.tensor.transpose(p_not[:], t_nog[:], t_eye[0:100, 0:100])
                    nc.scalar.copy(t_noT[:, g * 100:(g + 1) * 100], p_not[:])

                with tc.tile_pool(name="psn", bufs=1, space="PSUM") as psn:
                    p_irp = psn.tile([128, F], F32, tag="tmp", bufs=2)
                    nc.tensor.matmul(p_irp[:], t_air[:], t_nt[0:65, :], start=True, stop=True)
                    t_irp = big.tile([128, F], F32)
                    nc.vector.tensor_scalar(out=t_irp[:], in0=p_irp[:],
                                            scalar1=t_ccol[:, :], scalar2=None, op0=OP.add)
                    p_hre = psn.tile([128, F], F32, tag="tmp", bufs=2)
                    nc.tensor.matmul(p_hre[:], t_dcos[:], t_irp[:], start=True, stop=True)
                    t_hre = big.tile([128, F], F32)
                    nc.scalar.copy(t_hre[:], p_hre[:])
                    p_him = psn.tile([128, F], F32, tag="tmp", bufs=2)
                    nc.tensor.matmul(p_him[:], t_dsin[:], t_irp[:], start=True, stop=True)
                    t_him = big.tile([128, F], F32)
                    nc.scalar.copy(t_him[:], p_him[:])
                    p_h128 = psn.tile([1, F], F32, tag="tmp", bufs=2)
                    nc.tensor.matmul(p_h128[:], t_d128[:], t_irp[:], start=True, stop=True)
                    t_h128 = big.tile([1, F], F32)
                    nc.scalar.copy(t_h128[:], p_h128[:])
                    p_nre = psn.tile([128, F], F32, tag="nre")
                    nc.tensor.matmul(p_nre[:], t_dcos[:], t_noT[:], start=True, stop=True)
                    p_nim = psn.tile([128, F], F32, tag="nim")
                    nc.tensor.matmul(p_nim[:], t_dsin[:], t_noT[:], start=True, stop=True)
                    p_n128 = psn.tile([1, F], F32, tag="tmp", bufs=2)
                    nc.tensor.matmul(p_n128[:], t_d128[:], t_noT[:], start=True, stop=True)
                    t_a = work.tile([128, F], F32, tag="pa")
                    nc.vector.tensor_mul(t_a[:], t_hre[:], p_nre[:])
                    t_b = work.tile([128, F], F32, tag="pb")
                    nc.vector.tensor_mul(t_b[:], t_him[:], p_nim[:])
                    t_pre = big.tile([128, F], BF16)
                    nc.vector.tensor_sub(t_pre[:], t_a[:], t_b[:])
                    t_c2 = work.tile([128, F], F32, tag="pc2")
                    nc.vector.tensor_mul(t_c2[:], t_him[:], p_nre[:])
                    t_d2 = work.tile([128, F], F32, tag="pd2")
                    nc.vector.tensor_mul(t_d2[:], t_hre[:], p_nim[:])
                    t_pim = big.tile([128, F], BF16)
                    nc.vector.tensor_add(t_pim[:], t_c2[:], t_d2[:])
                    t_p128 = big.tile([1, F], BF16)
                    nc.vector.tensor_mul(t_p128[:], t_h128[:], p_n128[:])

            # ---------- harmonic stage B: depth-4 software pipeline ----------
            with tc.tile_pool(name="pskw", bufs=3, space="PSUM") as pskw, \
                 tc.tile_pool(name="psO", bufs=1, space="PSUM") as psO, \
                 tc.tile_pool(name="psnz2", bufs=1, space="PSUM") as psnz:
                p_nz = psnz.tile([128, F], F32)
                nc.tensor.matmul(p_nz[:], t_icre16[:], t_pre[:], start=True, stop=False)
                nc.tensor.matmul(p_nz[:], t_icim16[:], t_pim[:], start=False, stop=False)
                nc.tensor.matmul(p_nz[:], t_nyq16[:], t_p128[:], start=False, stop=True)

                p_O = psO.tile([128, F], F32)
                kws, t1s, svs, sns = {}, {}, {}, {}
                for c in range(NCHUNK + 4):
                    if c < NCHUNK:
                        t_wc = chk.tile([2, CHW], BF16, tag="wc")
                        nc.sync.dma_start(t_wc[:],
                                          bass.AP(wscr16, c * CHW, [[T, 2], [1, CHW]]))
                        p_kw = pskw.tile([128, CHW], F32, tag="kw")
                        for half in range(2):
                            nc.tensor.matmul(p_kw[:, half * 512:(half + 1) * 512],
                                             t_kv16[:], t_wc[:, half * 512:(half + 1) * 512],
                                             start=True, stop=True)
                        kws[c] = p_kw
                    if c - 1 >= 0 and c - 1 < NCHUNK:
                        cj = c - 1
                        pk = kws[cj]
                        t_t1 = chk.tile([128, CHW], F32, tag="t1")
                        if cj % 5 in (1, 3):
                            nc.vector.tensor_scalar(out=t_t1[:], in0=pk[:], scalar1=MAGIC,
                                                    scalar2=None, op0=OP.add)
                        else:
                            nc.scalar.activation(t_t1[:], pk[:], AF.Identity,
                                                 bias=t_mcol[:, :], scale=1.0)
                        t1s[cj] = t_t1
                    if c - 2 >= 0 and c - 2 < NCHUNK:
                        t_sv = chk.tile([128, CHW], F32, tag="sv")
                        nc.vector.scalar_tensor_tensor(out=t_sv[:], in0=t1s.pop(c - 2)[:],
                                                       scalar=MAGIC, in1=kws.pop(c - 2)[:],
                                                       op0=OP.subtract, op1=OP.subtract)
                        svs[c - 2] = t_sv
                    if c - 3 >= 0 and c - 3 < NCHUNK:
                        t_sn = chk.tile([128, CHW], F16, tag="sn")
                        nc.scalar.activation(t_sn[:], svs.pop(c - 3)[:], AF.Sin,
                                             scale=2.0 * math.pi)
                        sns[c - 3] = t_sn
                    if c - 4 >= 0:
                        cj = c - 4
                        t_sn = sns.pop(cj)
                        for fl in range(8):
                            f = 8 * cj + fl
                            nc.tensor.matmul(p_O[:, f:f + 1],
                                             t_sn[:, fl * 128:(fl + 1) * 128],
                                             t_at16[:, f:f + 1], start=True, stop=True)

                # ---------- combine (block-reversed samples) ----------
                t_spad16 = big.tile([128, 526], BF16)
                nc.vector.memset(t_spad16[:, 0:126], 0.0)
                nc.scalar.copy(t_spad16[:, 126:526], p_O[:])
                nc.vector.tensor_add(t_spad16[:, 126:526], t_spad16[:, 126:526], p_nz[:])

            # ---------- reverb: 126 accumulating bf16 Toeplitz matmuls ----------
            with tc.tile_pool(name="psrev", bufs=1, space="PSUM") as psr:
                p_rev = psr.tile([128, F], F32)
                for d in range(126):
                    nc.tensor.matmul(p_rev[:], t_hs16[:, d * 128:d * 128 + 128],
                                     t_spad16[:, 126 - d:526 - d],
                                     start=(d == 0), stop=(d == 125))
                t_out = big.tile([128, F], F32)
                nc.scalar.copy(t_out[:], p_rev[:])
                nc.sync.dma_start(out_d[:], t_out[:])

    nc.compile()
    return nc


def kernel(**inputs):
    if "nc" not in _cache:
        _cache["nc"] = _build()
        _cache["consts"] = _host_constants()
    nc = _cache["nc"]
    consts = _cache["consts"]

    amp = np.ascontiguousarray(np.asarray(inputs["amp_param"], np.float32))
    npr = np.ascontiguousarray(np.asarray(inputs["noise_param"], np.float32))
    pit = np.ascontiguousarray(np.asarray(inputs["pitch"], np.float32))
    noi = np.ascontiguousarray(np.asarray(inputs["noise"], np.float32))
    rvn = np.ascontiguousarray(np.asarray(inputs["reverb_noise"], np.float32))
    dec = np.asarray(inputs["decay"], np.float32).reshape(1, 1)
    wet = np.asarray(inputs["wet"], np.float32).reshape(1, 1)

    in_maps = []
    for b in range(B):
        m = dict(amp_param=amp[b], noise_param=npr[b], pitch=pit[b],
                 noise=noi[b], reverb_noise=rvn, decay=dec, wet=wet)
        m.update(consts)
        in_maps.append(m)

    res = run_bass_kernel_spmd(nc, in_maps, list(range(B)))
    out = np.stack([res.results[b]["out"].T.reshape(T, 1) for b in range(B)])
    return out.astype(np.float32)


if __name__ == "__main__":
    rng = np.random.default_rng(0)
    ins = dict(
        amp_param=rng.standard_normal((B, F, NH + 1)).astype(np.float32),
        noise_param=rng.standard_normal((B, F, NB)).astype(np.float32),
        pitch=(rng.random((B, F, 1), np.float32) * 440 + 60),
        noise=(rng.random((B, F, BLOCK), np.float32) * 2 - 1),
        reverb_noise=(rng.random((SR, 1), np.float32) * 2 - 1),
        decay=np.ones(1, np.float32) * 5,
        wet=np.zeros(1, np.float32),
        sampling_rate=SR, block_size=BLOCK,
    )
    o = kernel(**ins)
    print("kernel out", o.shape, o.dtype, np.abs(o).max())


def _install_ntff_hook():
    import sys as _sys
    import types as _types
    try:
        import antenv.axon_hooks  # noqa: F401
        return
    except ImportError:
        pass
    from trn_agent_boot.trn_boot import _ntff_profile_via_ctypes
    hook = _ntff_profile_via_ctypes('/opt/axon/libaxon_pjrt.so')
    mod = _types.ModuleType('antenv.axon_hooks')
    _h = {'v': hook}
    mod.get_axon_ntff_profile_hook = lambda: _h['v']
    mod.set_axon_ntff_profile_hook = lambda h: _h.update(v=h)
    _sys.modules['antenv.axon_hooks'] = mod
    import antenv
    antenv.axon_hooks = mod


def run_timed(**inputs):
    """Re-run with NTFF tracing enabled; returns max per-core exec ns or None."""
    _install_ntff_hook()
    if "nc" not in _cache:
        _cache["nc"] = _build()
        _cache["consts"] = _host_constants()
    nc = _cache["nc"]
    consts = _cache["consts"]
    amp = np.ascontiguousarray(np.asarray(inputs["amp_param"], np.float32))
    npr = np.ascontiguousarray(np.asarray(inputs["noise_param"], np.float32))
    pit = np.ascontiguousarray(np.asarray(inputs["pitch"], np.float32))
    noi = np.ascontiguousarray(np.asarray(inputs["noise"], np.float32))
    rvn = np.ascontiguousarray(np.asarray(inputs["reverb_noise"], np.float32))
    dec = np.asarray(inputs["decay"], np.float32).reshape(1, 1)
    wet = np.asarray(inputs["wet"], np.float32).reshape(1, 1)
    in_maps = []
    for b in range(B):
        m = dict(amp_param=amp[b], noise_param=npr[b], pitch=pit[b],
                 noise=noi[b], reverb_noise=rvn, decay=dec, wet=wet)
        m.update(consts)
        in_maps.append(m)
    res = run_bass_kernel_spmd(nc, in_maps, list(range(B)), trace=True)
    if res.instructions_and_trace is not None:
        _cache["insts"] = res.instructions_and_trace[0]
    return res.exec_time_ns


# revision 10
# speedup vs baseline: 3.7253x; 1.0337x over previous
# DDSP synthesizer kernel for Trainium2 (8 NeuronCores, batch-parallel).
#
# Per core (one batch element):
#   harmonic branch: exact-phase oscillator bank. Phases are wrapped to
#     [-0.5,0.5] in (j,f) layout, split hi/lo bf16, PE-transposed to
#     time-major, then k*w computed as a K=2 bf16 outer-product matmul
#     (exact products, fp32 PSUM accumulate). Magic-number wrap
#     (ScalarE/DVE split), ScalarE Sin, per-frame weighted reduction on PE.
#   noise branch: irfft+window folded into one DFT matmul, frame-wise
#     128-tap causal conv via rfft-256 as PE matmuls (inverse side bf16).
#   reverb: 16000-tap causal FIR as 126 accumulating bf16 Toeplitz matmuls;
#     the Toeplitz operand is fetched with 4 wide positive-stride DMAs
#     (samples are generated block-reversed so the shift structure needs
#     no negative strides).
import math
import numpy as np

import concourse.bass as bass
import concourse.bacc as bacc
import concourse.mybir as mybir
from concourse import tile
from concourse.bass_utils import run_bass_kernel_spmd

F32 = mybir.dt.float32
F16 = mybir.dt.float16
BF16 = mybir.dt.bfloat16
B, F, NH, NB = 8, 400, 100, 65
SR, BLOCK = 16000, 128
T = F * BLOCK
LOG10 = math.log(10.0)
MAGIC = 12582912.0  # 1.5 * 2**23
NCHUNK = 50         # harmonic chunks of 1024 samples
CHW = 1024

_cache = {}


def _host_constants():
    b = np.arange(NB)[:, None]
    m = np.arange(128)[None, :]
    w = np.where((b == 0) | (b == 64), 1.0, 2.0)
    Cmat = w / 128.0 * np.cos(2 * np.pi * b * m / 128.0)
    win2 = 0.5 + 0.5 * np.cos(2 * np.pi * np.arange(128) / 128.0)
    Air = (2.0 * Cmat * win2[None, :]).astype(np.float32)               # (65,128)
    ccol = (1e-7 * (Cmat * win2[None, :]).sum(0)).astype(np.float32)[:, None]  # (128,1)
    j = np.arange(128)[:, None]
    bb = np.arange(128)[None, :]
    Dcos = np.cos(2 * np.pi * j * bb / 256.0).astype(np.float32)        # (128j,128b)
    Dsin = (-np.sin(2 * np.pi * j * bb / 256.0)).astype(np.float32)
    d128 = np.cos(np.pi * np.arange(128)).astype(np.float32)[:, None]   # (128,1)
    bb2 = np.arange(128)[:, None]
    i = np.arange(128)[None, :]
    cb = np.where(bb2 == 0, 1.0, 2.0)
    ICre = (cb / 256.0 * np.cos(2 * np.pi * bb2 * i / 256.0)).astype(np.float32)
    ICim = (-2.0 / 256.0 * np.sin(2 * np.pi * bb2 * i / 256.0)).astype(np.float32)
    nyq = ((1.0 / 256.0) * np.cos(np.pi * np.arange(128))).astype(np.float32)[None, :]
    # output samples are produced block-reversed (partition p = sample 127-p)
    ICre = np.ascontiguousarray(ICre[:, ::-1])
    ICim = np.ascontiguousarray(ICim[:, ::-1])
    nyq = np.ascontiguousarray(nyq[:, ::-1])
    kvneg = np.zeros((1, 128), np.float32)
    kvneg[0, :NH] = -np.arange(1, NH + 1)
    eye = np.eye(128, dtype=np.float32)
    return dict(c_air=Air, c_ccol=ccol, c_dcos=Dcos, c_dsin=Dsin, c_d128=d128,
                c_icre=ICre, c_icim=ICim, c_nyq=nyq, c_kvneg=kvneg, c_eye=eye)


def _build():
    nc = bacc.Bacc(None, target_bir_lowering=False, debug=False)

    amp_d = nc.dram_tensor("amp_param", [F, NH + 1], F32, kind="ExternalInput")
    npr_d = nc.dram_tensor("noise_param", [F, NB], F32, kind="ExternalInput")
    pit_d = nc.dram_tensor("pitch", [F, 1], F32, kind="ExternalInput")
    noi_d = nc.dram_tensor("noise", [F, BLOCK], F32, kind="ExternalInput")
    rvn_d = nc.dram_tensor("reverb_noise", [SR, 1], F32, kind="ExternalInput")
    dec_d = nc.dram_tensor("decay", [1, 1], F32, kind="ExternalInput")
    wet_d = nc.dram_tensor("wet", [1, 1], F32, kind="ExternalInput")
    cst = {}
    for name, shape in [("c_air", [NB, 128]), ("c_ccol", [128, 1]),
                        ("c_dcos", [128, 128]), ("c_dsin", [128, 128]),
                        ("c_d128", [128, 1]), ("c_icre", [128, 128]),
                        ("c_icim", [128, 128]), ("c_nyq", [1, 128]),
                        ("c_kvneg", [1, 128]), ("c_eye", [128, 128])]:
        cst[name] = nc.dram_tensor(name, shape, F32, kind="ExternalInput")
    out_d = nc.dram_tensor("out", [128, F], F32, kind="ExternalOutput")

    wscr16 = nc.dram_tensor("wscr16", [2, T], BF16)
    hpbuf16 = nc.dram_tensor("hpbuf16", [1, 16384], BF16)

    AF = mybir.ActivationFunctionType
    OP = mybir.AluOpType

    with tile.TileContext(nc) as tc:
        with tc.tile_pool(name="const", bufs=1) as cpool, \
             tc.tile_pool(name="big", bufs=1) as big, \
             tc.tile_pool(name="work", bufs=1) as work, \
             tc.tile_pool(name="chunk", bufs=3) as chk:

            # ---------- constants ----------
            t_mcol = cpool.tile([128, 1], F32)
            nc.vector.memset(t_mcol[:], MAGIC)
            t_eps = cpool.tile([128, 1], F32)
            nc.vector.memset(t_eps[:], 1e-7)
            t_b5 = cpool.tile([128, 1], F32)
            nc.vector.memset(t_b5[:], 5.0)
            t_kv2f = cpool.tile([2, 128], F32)
            nc.sync.dma_start(t_kv2f[:], bass.AP(cst["c_kvneg"], 0, [[0, 2], [1, 128]]))
            t_kv16 = cpool.tile([2, 128], BF16)
            nc.vector.tensor_copy(t_kv16[:], t_kv2f[:])
            t_air = cpool.tile([NB, 128], F32)
            nc.sync.dma_start(t_air[:], cst["c_air"][:])
            t_ccol = cpool.tile([128, 1], F32)
            nc.sync.dma_start(t_ccol[:], cst["c_ccol"][:])
            t_dcos = cpool.tile([128, 128], F32)
            nc.sync.dma_start(t_dcos[:], cst["c_dcos"][:])
            t_dsin = cpool.tile([128, 128], F32)
            nc.sync.dma_start(t_dsin[:], cst["c_dsin"][:])
            t_d128 = cpool.tile([128, 1], F32)
            nc.sync.dma_start(t_d128[:], cst["c_d128"][:])
            t_icre32 = cpool.tile([128, 128], F32)
            nc.sync.dma_start(t_icre32[:], cst["c_icre"][:])
            t_icre16 = cpool.tile([128, 128], BF16)
            nc.vector.tensor_copy(t_icre16[:], t_icre32[:])
            t_icim32 = cpool.tile([128, 128], F32)
            nc.sync.dma_start(t_icim32[:], cst["c_icim"][:])
            t_icim16 = cpool.tile([128, 128], BF16)
            nc.vector.tensor_copy(t_icim16[:], t_icim32[:])
            t_nyq32 = cpool.tile([1, 128], F32)
            nc.sync.dma_start(t_nyq32[:], cst["c_nyq"][:])
            t_nyq16 = cpool.tile([1, 128], BF16)
            nc.vector.tensor_copy(t_nyq16[:], t_nyq32[:])
            t_eye = cpool.tile([128, 128], F32)
            nc.sync.dma_start(t_eye[:], cst["c_eye"][:])
            t_eye16 = cpool.tile([128, 128], BF16)
            nc.vector.tensor_copy(t_eye16[:], t_eye[:])
            t_kroi = cpool.tile([128, NH], mybir.dt.int32)
            nc.gpsimd.iota(t_kroi[:], pattern=[[1, NH]], base=1, channel_multiplier=0)
            t_krow = cpool.tile([128, NH], F32)
            nc.vector.tensor_copy(t_krow[:], t_kroi[:])
            t_jp1i = cpool.tile([128, 1], mybir.dt.int32)
            nc.gpsimd.iota(t_jp1i[:], pattern=[[1, 1]], base=1, channel_multiplier=1)
            t_jp1 = cpool.tile([128, 1], F32)
            nc.vector.tensor_copy(t_jp1[:], t_jp1i[:])
            t_jrev = cpool.tile([128, 1], F32)
            nc.vector.tensor_scalar(out=t_jrev[:], in0=t_jp1[:], scalar1=-1.0,
                                    scalar2=129.0, op0=OP.mult, op1=OP.add)
            t_ioti = cpool.tile([128, 125], mybir.dt.int32)
            nc.gpsimd.iota(t_ioti[:], pattern=[[128, 125]], base=0, channel_multiplier=1)
            t_iotf = cpool.tile([128, 125], F32)
            nc.vector.tensor_copy(t_iotf[:], t_ioti[:])

            # early loads for impulse build
            t_dec = work.tile([1, 1], F32)
            nc.sync.dma_start(t_dec[:], dec_d[:])
            t_wet = work.tile([1, 1], F32)
            nc.sync.dma_start(t_wet[:], wet_d[:])
            t_rn = work.tile([128, 125], F32)
            nc.sync.dma_start(t_rn[:], bass.AP(rvn_d, 0, [[1, 128], [128, 125]]))

            with tc.high_priority():
                # ---------- phase chain (critical path to stage B) ----------
                t_pr = work.tile([1, F], F32)
                nc.sync.dma_start(t_pr[:], bass.AP(pit_d, 0, [[1, 1], [1, F]]))
                t_p8 = work.tile([1, F], F32)
                nc.scalar.activation(t_p8[:], t_pr[:], AF.Identity,
                                     bias=t_mcol[0:1, :], scale=8.0)
                t_ph = work.tile([1, F], F32)
                nc.vector.tensor_scalar(out=t_ph[:], in0=t_p8[:], scalar1=MAGIC,
                                        scalar2=0.125, op0=OP.subtract, op1=OP.mult)
                t_pl = work.tile([1, F], F32)
                nc.vector.tensor_sub(t_pl[:], t_pr[:], t_ph[:])
                t_zr = work.tile([1, F], F32)
                nc.vector.memset(t_zr[:], 0.0)
                t_sh = work.tile([1, F + 1], F32)
                nc.vector.memset(t_sh[:, 0:1], 0.0)
                nc.vector.tensor_tensor_scan(t_sh[:, 1:F + 1], t_ph[:], t_zr[:], 0.0,
                                             OP.add, OP.add)
                t_sl = work.tile([1, F + 1], F32)
                nc.vector.memset(t_sl[:, 0:1], 0.0)
                nc.vector.tensor_tensor_scan(t_sl[:, 1:F + 1], t_pl[:], t_zr[:], 0.0,
                                             OP.add, OP.add)
                t_ts = work.tile([1, F], F32)
                nc.vector.tensor_add(t_ts[:], t_sh[:, 0:F], t_sl[:, 0:F])
                t_t2r = work.tile([1, F], F32)
                nc.scalar.activation(t_t2r[:], t_ts[:], AF.Identity,
                                     bias=t_mcol[0:1, :], scale=1.0 / 125.0)
                t_n125 = work.tile([1, F], F32)
                nc.vector.tensor_scalar(out=t_n125[:], in0=t_t2r[:], scalar1=MAGIC,
                                        scalar2=None, op0=OP.subtract)
                t_u = work.tile([1, F], F32)
                nc.vector.scalar_tensor_tensor(out=t_u[:], in0=t_n125[:], scalar=-125.0,
                                               in1=t_sh[:, 0:F], op0=OP.mult, op1=OP.add)
                t_r125 = work.tile([1, F], F32)
                nc.vector.tensor_add(t_r125[:], t_u[:], t_sl[:, 0:F])
                t_om = work.tile([1, F], F32)
                nc.vector.tensor_scalar(out=t_om[:], in0=t_r125[:], scalar1=1.0 / 125.0,
                                        scalar2=None, op0=OP.mult)
                t_cr = work.tile([1, F], F32)
                nc.vector.tensor_scalar(out=t_cr[:], in0=t_pr[:], scalar1=1.0 / 16000.0,
                                        scalar2=None, op0=OP.mult)
                t_omb = work.tile([128, F], F32)
                nc.gpsimd.partition_broadcast(t_omb[:], t_om[:])
                t_cb = work.tile([128, F], F32)
                nc.gpsimd.partition_broadcast(t_cb[:], t_cr[:])
                t_wraw = work.tile([128, F], F32)
                nc.vector.scalar_tensor_tensor(out=t_wraw[:], in0=t_cb[:],
                                               scalar=t_jrev[:, :], in1=t_omb[:],
                                               op0=OP.mult, op1=OP.add)
                t_tw = work.tile([128, F], F32)
                nc.scalar.activation(t_tw[:], t_wraw[:], AF.Identity,
                                     bias=t_mcol[:, :], scale=1.0)
                t_wneg = work.tile([128, F], F32)
                nc.vector.scalar_tensor_tensor(out=t_wneg[:], in0=t_tw[:], scalar=MAGIC,
                                               in1=t_wraw[:], op0=OP.subtract,
                                               op1=OP.subtract)
                t_whi16 = work.tile([128, F], BF16)
                nc.vector.tensor_copy(t_whi16[:], t_wneg[:])
                t_wlo16 = work.tile([128, F], BF16)
                nc.vector.tensor_sub(t_wlo16[:], t_wneg[:], t_whi16[:])
                with tc.tile_pool(name="pswt", bufs=2, space="PSUM") as pswt:
                    t_wThi = work.tile([100, 512], BF16)
                    t_wTlo = work.tile([100, 512], BF16)
                    for g in range(4):
                        p_th = pswt.tile([100, 128], BF16, tag="wt")
                        nc.tensor.transpose(p_th[:], t_whi16[:, g * 100:(g + 1) * 100],
                                            t_eye16[:])
                        nc.scalar.copy(t_wThi[:, g * 128:(g + 1) * 128], p_th[:])
                        p_tl = pswt.tile([100, 128], BF16, tag="wt")
                        nc.tensor.transpose(p_tl[:], t_wlo16[:, g * 100:(g + 1) * 100],
                                            t_eye16[:])
                        nc.scalar.copy(t_wTlo[:, g * 128:(g + 1) * 128], p_tl[:])
                    # group 0 first (unblocks chunk 0), then groups 1-3 merged;
                    # hi on sync queue, lo on gpsimd queue, in parallel
                    nc.sync.dma_start(
                        bass.AP(wscr16, 0, [[128, 100], [1, 128]]),
                        t_wThi[:, 0:128])
                    nc.gpsimd.dma_start(
                        bass.AP(wscr16, T, [[128, 100], [1, 128]]),
                        t_wTlo[:, 0:128])
                    nc.sync.dma_start(
                        bass.AP(wscr16, 12800, [[128, 100], [12800, 3], [1, 128]]),
                        t_wThi[:, 128:512])
                    nc.gpsimd.dma_start(
                        bass.AP(wscr16, T + 12800, [[128, 100], [12800, 3], [1, 128]]),
                        t_wTlo[:, 128:512])

            # ---------- impulse + Toeplitz gather (gpsimd queue) ----------
            t_e1 = work.tile([1, 1], F32)
            nc.scalar.activation(t_e1[:], t_dec[:], AF.Exp, scale=-1.0)
            t_sp = work.tile([1, 1], F32)
            nc.scalar.activation(t_sp[:], t_e1[:], AF.Ln, bias=1.0, scale=1.0)
            t_s32 = work.tile([1, 1], F32)
            nc.vector.tensor_scalar(out=t_s32[:], in0=t_sp[:], scalar1=-1.0 / 32.0,
                                    scalar2=None, op0=OP.mult)
            t_ew = work.tile([1, 1], F32)
            nc.scalar.activation(t_ew[:], t_wet[:], AF.Exp, scale=-1.0)
            t_ew1 = work.tile([1, 1], F32)
            nc.vector.tensor_scalar(out=t_ew1[:], in0=t_ew[:], scalar1=1.0,
                                    scalar2=None, op0=OP.add)
            t_sw = work.tile([1, 1], F32)
            nc.vector.reciprocal(t_sw[:], t_ew1[:])
            t_s32b = work.tile([128, 1], F32)
            nc.gpsimd.partition_broadcast(t_s32b[:], t_s32[:])
            t_swb = work.tile([128, 1], F32)
            nc.gpsimd.partition_broadcast(t_swb[:], t_sw[:])
            t_env = work.tile([128, 125], F32)
            nc.scalar.activation(t_env[:], t_iotf[:], AF.Exp, scale=t_s32b[:, :])
            t_h = work.tile([128, 125], F32)
            nc.vector.scalar_tensor_tensor(out=t_h[:], in0=t_env[:], scalar=t_swb[:, :],
                                           in1=t_rn[:], op0=OP.mult, op1=OP.mult)
            nc.vector.memset(t_h[0:1, 0:1], 1.0)
            with tc.tile_pool(name="psimp", bufs=1, space="PSUM") as psimp:
                p_ht = psimp.tile([125, 128], F32)
                nc.tensor.transpose(p_ht[:], t_h[:], t_eye[:])
                t_hT16 = work.tile([125, 128], BF16)
                nc.scalar.copy(t_hT16[:], p_ht[:])
            t_z16 = work.tile([1, 160], BF16)
            nc.vector.memset(t_z16[:], 0.0)
            nc.gpsimd.dma_start(bass.AP(hpbuf16, 0, [[1, 1], [1, 127]]),
                                t_z16[0:1, 0:127])
            nc.gpsimd.dma_start(bass.AP(hpbuf16, 127, [[128, 125], [1, 128]]),
                                t_hT16[:])
            nc.gpsimd.dma_start(bass.AP(hpbuf16, 16127, [[1, 1], [1, 129]]),
                                t_z16[0:1, 0:129])
            t_hs16 = big.tile([128, 16128], BF16)
            for q in range(4):
                nc.gpsimd.dma_start(
                    t_hs16[q * 32:(q + 1) * 32, :],
                    bass.AP(hpbuf16, q * 32, [[1, 32], [128, 126], [1, 128]]))

            with tc.tile_pool(name="pstr", bufs=2, space="PSUM") as pstr:
                # ---------- amp params ----------
                t_at = big.tile([128, F], F32)
                nc.vector.memset(t_at[:], 0.0)
                amp_sf = []
                for g in range(4):
                    f0 = g * 100
                    t_ap = work.tile([100, NH + 1], F32, tag=f"ampchunk{g}", name=f"ampchunk{g}")
                    nc.sync.dma_start(t_ap[:], amp_d[f0:f0 + 100, :])
                    t_sf = work.tile([100, NH + 1], F32, tag=f"ampsf{g}", name=f"ampsf{g}")
                    nc.scalar.activation(t_sf[:], t_ap[:], AF.Exp, scale=-1.0)
                    amp_sf.append(t_sf)
                ns_sf = []
                for g in range(4):
                    f0 = g * 100
                    t_np = work.tile([100, NB], F32, tag=f"npchunk{g}", name=f"npchunk{g}")
                    nc.sync.dma_start(t_np[:], npr_d[f0:f0 + 100, :])
                    t_ns = work.tile([100, NB], F32, tag=f"nsf{g}", name=f"nsf{g}")
                    nc.scalar.activation(t_ns[:], t_np[:], AF.Exp, bias=t_b5[0:100, :], scale=-1.0)
                    ns_sf.append(t_ns)
                for t_sf in amp_sf + ns_sf:
                    nc.scalar.activation(t_sf[:], t_sf[:], AF.Ln, bias=1.0, scale=1.0)
                for t_sf in amp_sf + ns_sf:
                    nc.scalar.activation(t_sf[:], t_sf[:], AF.Exp, scale=-LOG10)
                for t_sf in amp_sf:
                    nc.scalar.activation(t_sf[:], t_sf[:], AF.Identity,
                                         bias=t_eps[0:100, :], scale=2.0)
                t_pc4 = work.tile([4, 100], F32)
                nc.sync.dma_start(t_pc4[:], bass.AP(pit_d, 0, [[100, 4], [1, 100]]))
                p_pc = pstr.tile([100, 4], F32, tag="tr")
                nc.tensor.transpose(p_pc[:], t_pc4[:], t_eye[0:4, 0:4])
                t_pcol = work.tile([100, 4], F32)
                nc.scalar.copy(t_pcol[:], p_pc[:])
                for g in range(4):
                    f0 = g * 100
                    t_sf = amp_sf[g]
                    t_kp = work.tile([100, NH], F32, tag="kp")
                    nc.vector.tensor_scalar(out=t_kp[:], in0=t_krow[0:100, :],
                                            scalar1=t_pcol[:, g:g + 1], scalar2=None, op0=OP.mult)
                    t_aa = work.tile([100, NH], F32, tag="aa")
                    nc.vector.tensor_scalar(out=t_aa[:], in0=t_kp[:], scalar1=8000.0,
                                            scalar2=1e-4, op0=OP.is_lt, op1=OP.add)
                    t_am = work.tile([100, NH], F32, tag="am")
                    nc.vector.tensor_mul(t_am[:], t_sf[:, 1:NH + 1], t_aa[:])
                    t_ssum = work.tile([100, 1], F32, tag="ssum")
                    nc.vector.tensor_reduce(out=t_ssum[:], in_=t_am[:],
                                            axis=mybir.AxisListType.X, op=OP.add)
                    t_rec = work.tile([100, 1], F32, tag="rec")
                    nc.vector.reciprocal(t_rec[:], t_ssum[:])
                    t_scn = work.tile([100, 1], F32, tag="scn")
                    nc.vector.tensor_mul(t_scn[:], t_rec[:], t_sf[:, 0:1])
                    nc.vector.tensor_scalar(out=t_scn[:], in0=t_scn[:], scalar1=-1.0,
                                            scalar2=None, op0=OP.mult)
                    nc.vector.tensor_scalar(out=t_am[:], in0=t_am[:], scalar1=t_scn[:, :],
                                            scalar2=None, op0=OP.mult)
                    p_tr = pstr.tile([100, 100], F32, tag="tr")
                    nc.tensor.transpose(p_tr[:], t_am[:], t_eye[0:100, 0:100])
                    nc.scalar.copy(t_at[0:100, f0:f0 + 100], p_tr[:])
                t_at16 = big.tile([128, F], F16)
                nc.vector.tensor_copy(t_at16[:], t_at[:])

                # ---------- noise branch ----------
                t_nt = big.tile([65, F], F32)
                for g in range(4):
                    f0 = g * 100
                    p_tr2 = pstr.tile([65, 100], F32, tag="tr")
                    nc.tensor.transpose(p_tr2[:], ns_sf[g][:], t_eye[0:100, 0:100])
                    nc.scalar.copy(t_nt[0:65, f0:f0 + 100], p_tr2[:])
                t_noT = big.tile([128, F], F32)
                for g in range(4):
                    t_nog = work.tile([100, 128], F32, tag=f"nog{g}", name=f"nog{g}")
                    nc.sync.dma_start(t_nog[:], noi_d[g * 100:(g + 1) * 100, :])
                    p_not = pstr.tile([128, 100], F32, tag="tr")
                    nc.tensor.transpose(p_not[:], t_nog[:], t_eye[0:100, 0:100])
                    nc.scalar.copy(t_noT[:, g * 100:(g + 1) * 100], p_not[:])

                with tc.tile_pool(name="psn", bufs=1, space="PSUM") as psn:
                    p_irp = psn.tile([128, F], F32, tag="tmp", bufs=2)
                    nc.tensor.matmul(p_irp[:], t_air[:], t_nt[0:65, :], start=True, stop=True)
                    t_irp = big.tile([128, F], F32)
                    nc.vector.tensor_scalar(out=t_irp[:], in0=p_irp[:],
                                            scalar1=t_ccol[:, :], scalar2=None, op0=OP.add)
                    p_hre = psn.tile([128, F], F32, tag="tmp", bufs=2)
                    nc.tensor.matmul(p_hre[:], t_dcos[:], t_irp[:], start=True, stop=True)
                    t_hre = big.tile([128, F], F32)
                    nc.scalar.copy(t_hre[:], p_hre[:])
                    p_him = psn.tile([128, F], F32, tag="tmp", bufs=2)
                    nc.tensor.matmul(p_him[:], t_dsin[:], t_irp[:], start=True, stop=True)
                    t_him = big.tile([128, F], F32)
                    nc.scalar.copy(t_him[:], p_him[:])
                    p_h128 = psn.tile([1, F], F32, tag="tmp", bufs=2)
                    nc.tensor.matmul(p_h128[:], t_d128[:], t_irp[:], start=True, stop=True)
                    t_h128 = big.tile([1, F], F32)
                    nc.scalar.copy(t_h128[:], p_h128[:])
                    p_nre = psn.tile([128, F], F32, tag="nre")
                    nc.tensor.matmul(p_nre[:], t_dcos[:], t_noT[:], start=True, stop=True)
                    p_nim = psn.tile([128, F], F32, tag="nim")
                    nc.tensor.matmul(p_nim[:], t_dsin[:], t_noT[:], start=True, stop=True)
                    p_n128 = psn.tile([1, F], F32, tag="tmp", bufs=2)
                    nc.tensor.matmul(p_n128[:], t_d128[:], t_noT[:], start=True, stop=True)
                    t_a = work.tile([128, F], F32, tag="pa")
                    nc.vector.tensor_mul(t_a[:], t_hre[:], p_nre[:])
                    t_b = work.tile([128, F], F32, tag="pb")
                    nc.vector.tensor_mul(t_b[:], t_him[:], p_nim[:])
                    t_pre = big.tile([128, F], BF16)
                    nc.vector.tensor_sub(t_pre[:], t_a[:], t_b[:])
                    t_c2 = work.tile([128, F], F32, tag="pc2")
                    nc.vector.tensor_mul(t_c2[:], t_him[:], p_nre[:])
                    t_d2 = work.tile([128, F], F32, tag="pd2")
                    nc.vector.tensor_mul(t_d2[:], t_hre[:], p_nim[:])
                    t_pim = big.tile([128, F], BF16)
                    nc.vector.tensor_add(t_pim[:], t_c2[:], t_d2[:])
                    t_p128 = big.tile([1, F], BF16)
                    nc.vector.tensor_mul(t_p128[:], t_h128[:], p_n128[:])

            # ---------- noise into signal tile (frees PSUM for reverb) ----------
            t_spad16 = big.tile([128, 526], BF16)
            with tc.tile_pool(name="psnz2", bufs=1, space="PSUM") as psnz:
                p_nz = psnz.tile([128, F], F32)
                nc.tensor.matmul(p_nz[:], t_icre16[:], t_pre[:], start=True, stop=False)
                nc.tensor.matmul(p_nz[:], t_icim16[:], t_pim[:], start=False, stop=False)
                nc.tensor.matmul(p_nz[:], t_nyq16[:], t_p128[:], start=False, stop=True)
                nc.vector.memset(t_spad16[:, 0:126], 0.0)
                nc.scalar.copy(t_spad16[:, 126:526], p_nz[:])

            # ---------- harmonic stage B + overlapped reverb ----------
            with tc.tile_pool(name="pskw", bufs=3, space="PSUM") as pskw, \
                 tc.tile_pool(name="psO", bufs=1, space="PSUM") as psO, \
                 tc.tile_pool(name="psrev", bufs=1, space="PSUM") as psr:
                p_rev = psr.tile([128, F], F32)
                p_O = psO.tile([128, F], F32)
                taps_launched = 0
                kws, t1s, svs, sns = {}, {}, {}, {}
                for c in range(NCHUNK + 4):
                    if c < NCHUNK:
                        t_wc = chk.tile([2, CHW], BF16, tag="wc")
                        nc.sync.dma_start(t_wc[:],
                                          bass.AP(wscr16, c * CHW, [[T, 2], [1, CHW]]))
                        p_kw = pskw.tile([128, CHW], F32, tag="kw")
                        for half in range(2):
                            nc.tensor.matmul(p_kw[:, half * 512:(half + 1) * 512],
                                             t_kv16[:], t_wc[:, half * 512:(half + 1) * 512],
                                             start=True, stop=True)
                        kws[c] = p_kw
                    if c - 1 >= 0 and c - 1 < NCHUNK:
                        cj = c - 1
                        pk = kws[cj]
                        t_t1 = chk.tile([128, CHW], F32, tag="t1")
                        if cj % 5 in (1, 3):
                            nc.vector.tensor_scalar(out=t_t1[:], in0=pk[:], scalar1=MAGIC,
                                                    scalar2=None, op0=OP.add)
                        else:
                            nc.scalar.activation(t_t1[:], pk[:], AF.Identity,
                                                 bias=t_mcol[:, :], scale=1.0)
                        t1s[cj] = t_t1
                    if c - 2 >= 0 and c - 2 < NCHUNK:
                        t_sv = chk.tile([128, CHW], F32, tag="sv")
                        nc.vector.scalar_tensor_tensor(out=t_sv[:], in0=t1s.pop(c - 2)[:],
                                                       scalar=MAGIC, in1=kws.pop(c - 2)[:],
                                                       op0=OP.subtract, op1=OP.subtract)
                        svs[c - 2] = t_sv
                    if c - 3 >= 0 and c - 3 < NCHUNK:
                        t_sn = chk.tile([128, CHW], F16, tag="sn")
                        nc.scalar.activation(t_sn[:], svs.pop(c - 3)[:], AF.Sin,
                                             scale=2.0 * math.pi)
                        sns[c - 3] = t_sn
                    if c - 4 >= 0:
                        cj = c - 4
                        t_sn = sns.pop(cj)
                        for fl in range(8):
                            f = 8 * cj + fl
                            nc.tensor.matmul(p_O[:, f:f + 1],
                                             t_sn[:, fl * 128:(fl + 1) * 128],
                                             t_at16[:, f:f + 1], start=True, stop=True)
                        if cj == 24:
                            # first half of the signal is final: fold into spad
                            nc.vector.tensor_add(t_spad16[:, 126:326],
                                                 t_spad16[:, 126:326], p_O[:, 0:200])
                        if cj >= 24 and taps_launched < 126:
                            # reverb pass 1 (output frames 0:200), ~7 taps/chunk
                            nhere = min(7, 126 - taps_launched)
                            for d in range(taps_launched, taps_launched + nhere):
                                nc.tensor.matmul(p_rev[:, 0:200],
                                                 t_hs16[:, d * 128:d * 128 + 128],
                                                 t_spad16[:, 126 - d:326 - d],
                                                 start=(d == 0), stop=(d == 125))
                            taps_launched += nhere

                # ---------- second half: fold + reverb pass 2 ----------
                nc.vector.tensor_add(t_spad16[:, 326:526],
                                     t_spad16[:, 326:526], p_O[:, 200:400])
                for d in range(126):
                    nc.tensor.matmul(p_rev[:, 200:400],
                                     t_hs16[:, d * 128:d * 128 + 128],
                                     t_spad16[:, 326 - d:526 - d],
                                     start=(d == 0), stop=(d == 125))
                t_out = big.tile([128, F], F32)
                nc.scalar.copy(t_out[:], p_rev[:])
                nc.sync.dma_start(out_d[:], t_out[:])

    nc.compile()
    return nc


def kernel(**inputs):
    if "nc" not in _cache:
        _cache["nc"] = _build()
        _cache["consts"] = _host_constants()
    nc = _cache["nc"]
    consts = _cache["consts"]

    amp = np.ascontiguousarray(np.asarray(inputs["amp_param"], np.float32))
    npr = np.ascontiguousarray(np.asarray(inputs["noise_param"], np.float32))
    pit = np.ascontiguousarray(np.asarray(inputs["pitch"], np.float32))
    noi = np.ascontiguousarray(np.asarray(inputs["noise"], np.float32))
    rvn = np.ascontiguousarray(np.asarray(inputs["reverb_noise"], np.float32))
    dec = np.asarray(inputs["decay"], np.float32).reshape(1, 1)
    wet = np.asarray(inputs["wet"], np.float32).reshape(1, 1)

    in_maps = []
    for b in range(B):
        m = dict(amp_param=amp[b], noise_param=npr[b], pitch=pit[b],
                 noise=noi[b], reverb_noise=rvn, decay=dec, wet=wet)
        m.update(consts)
        in_maps.append(m)

    res = run_bass_kernel_spmd(nc, in_maps, list(range(B)))
    out = np.stack([res.results[b]["out"].T.reshape(T, 1) for b in range(B)])
    return out.astype(np.float32)


if __name__ == "__main__":
    rng = np.random.default_rng(0)
    ins = dict(
        amp_param=rng.standard_normal((B, F, NH + 1)).astype(np.float32),
        noise_param=rng.standard_normal((B, F, NB)).astype(np.float32),
        pitch=(rng.random((B, F, 1), np.float32) * 440 + 60),
        noise=(rng.random((B, F, BLOCK), np.float32) * 2 - 1),
        reverb_noise=(rng.random((SR, 1), np.float32) * 2 - 1),
        decay=np.ones(1, np.float32) * 5,
        wet=np.zeros(1, np.float32),
        sampling_rate=SR, block_size=BLOCK,
    )
    o = kernel(**ins)
    print("kernel out", o.shape, o.dtype, np.abs(o).max())


def _install_ntff_hook():
    import sys as _sys
    import types as _types
    try:
        import antenv.axon_hooks  # noqa: F401
        return
    except ImportError:
        pass
    from trn_agent_boot.trn_boot import _ntff_profile_via_ctypes
    hook = _ntff_profile_via_ctypes('/opt/axon/libaxon_pjrt.so')
    mod = _types.ModuleType('antenv.axon_hooks')
    _h = {'v': hook}
    mod.get_axon_ntff_profile_hook = lambda: _h['v']
    mod.set_axon_ntff_profile_hook = lambda h: _h.update(v=h)
    _sys.modules['antenv.axon_hooks'] = mod
    import antenv
    antenv.axon_hooks = mod


def run_timed(**inputs):
    """Re-run with NTFF tracing enabled; returns max per-core exec ns or None."""
    _install_ntff_hook()
    if "nc" not in _cache:
        _cache["nc"] = _build()
        _cache["consts"] = _host_constants()
    nc = _cache["nc"]
    consts = _cache["consts"]
    amp = np.ascontiguousarray(np.asarray(inputs["amp_param"], np.float32))
    npr = np.ascontiguousarray(np.asarray(inputs["noise_param"], np.float32))
    pit = np.ascontiguousarray(np.asarray(inputs["pitch"], np.float32))
    noi = np.ascontiguousarray(np.asarray(inputs["noise"], np.float32))
    rvn = np.ascontiguousarray(np.asarray(inputs["reverb_noise"], np.float32))
    dec = np.asarray(inputs["decay"], np.float32).reshape(1, 1)
    wet = np.asarray(inputs["wet"], np.float32).reshape(1, 1)
    in_maps = []
    for b in range(B):
        m = dict(amp_param=amp[b], noise_param=npr[b], pitch=pit[b],
                 noise=noi[b], reverb_noise=rvn, decay=dec, wet=wet)
        m.update(consts)
        in_maps.append(m)
    res = run_bass_kernel_spmd(nc, in_maps, list(range(B)), trace=True)
    if res.instructions_and_trace is not None:
        _cache["insts"] = res.instructions_and_trace[0]
    return res.exec_time_ns
